# revision 18
# baseline (speedup 1.0000x reference)
"""Trainium2 Bass kernel for a pre-norm transformer block (B=2, N=2048, D=1024, H=16, FF=4096).

Strategy (8 cores):
  Phase 1 (DP on batch x TP on heads): cores 0-3 handle batch 0, cores 4-7 batch 1;
    each core computes LN1 + qkv + attention for its 4 heads over all 2048 tokens,
    in feature-major ("transposed") layout so no on-chip transposes are needed.
    LN1 is folded into the weights host-side (gamma-scaling + mean-centering of the
    qkv weight columns); the per-token rstd is applied after the matmul via a
    DMA-broadcast row. Softmax runs without max-subtraction (scores are O(10), fp32
    exp is safe); denominators come from column-packed ones-matmuls.
  Reshard: one AllToAll per 4-core group moves head-shards -> token-shards.
  Phase 2 (token-parallel): each core runs proj + residual + LN2 + MLP for its 512
    tokens with full weights. proj_b is pre-added to the residual host-side; ln2_g/b
    are folded into fc1_w/fc1_b host-side; fc2_b is added host-side after gather.

All matmuls run in float32r (full PE rate at N>=256, ~1e-3 matmul accuracy).
"""

import numpy as np

DIM = 1024
HEADS = 16
HD = 64
FF = 4096
B = 2
N = 2048
EPS = 1e-5
P = 128
NCORES = 8
GROUPS = [[0, 1, 2, 3], [4, 5, 6, 7]]
TOKS = 512         # tokens per core in phase 2
C8 = DIM // P      # 8 contraction chunks
NT = N // 512      # 4 n-tiles
MC16 = N // P      # 16 m-chunks
FF32 = FF // P     # 32 ff chunks

_CACHE = {}


def _build():
    if "nc" in _CACHE:
        return _CACHE["nc"]

    import concourse.bacc as bacc
    import concourse.bass as bass
    import concourse.tile as tile
    from concourse import mybir
    from concourse.masks import make_identity
    from contextlib import ExitStack

    f32 = mybir.dt.float32
    f32r = mybir.dt.float32r
    AF = mybir.ActivationFunctionType
    ALU = mybir.AluOpType

    nc = bacc.Bacc("TRN2", target_bir_lowering=False, debug=False,
                   num_devices=NCORES)

    # ---- per-core dram tensors ----
    xT_d = nc.dram_tensor("xT", [DIM, N], f32r, kind="ExternalInput")
    wqk_d = nc.dram_tensor("w_qk", [DIM, 512], f32r, kind="ExternalInput")
    wv_d = nc.dram_tensor("w_v", [DIM, 256], f32r, kind="ExternalInput")
    bqk_d = nc.dram_tensor("b_qk", [P, 4], f32, kind="ExternalInput")
    bv_d = nc.dram_tensor("b_v", [P, 2], f32, kind="ExternalInput")
    wp_d = nc.dram_tensor("w_p", [DIM, DIM], f32r, kind="ExternalInput")
    xpb_d = nc.dram_tensor("x_pb", [TOKS, DIM], f32, kind="ExternalInput")
    wf1_d = nc.dram_tensor("w_f1", [DIM, FF], f32r, kind="ExternalInput")
    bf1_d = nc.dram_tensor("b_f1", [P, FF32], f32, kind="ExternalInput")
    wf2_d = nc.dram_tensor("w_f2", [FF, DIM], f32r, kind="ExternalInput")
    ones_d = nc.dram_tensor("ones_c", [P, 32], f32r, kind="ExternalInput")
    yout_d = nc.dram_tensor("y_out", [TOKS, DIM], f32, kind="ExternalOutput")

    def bc_row(ap_row, parts):
        # partition-broadcast AP for DMA: read one row into `parts` partitions
        t = ap_row
        dims = [list(d) for d in t.ap]
        if dims and dims[0][1] == 1:
            dims = dims[1:]
        return bass.AP(tensor=t.tensor, offset=t.offset,
                       ap=[[0, parts]] + dims)

    with tile.TileContext(nc) as tc:
        with ExitStack() as top:
            const = top.enter_context(tc.tile_pool(name="const", bufs=1))
            ones_r = const.tile([P, 32], f32r, name="ones_r")
            nc.sync.dma_start(out=ones_r, in_=ones_d.ap())
            ident = const.tile([P, P], f32, name="ident")
            make_identity(nc, ident)
            bqk_sb = const.tile([P, 4], f32, name="bqk_sb")
            nc.sync.dma_start(out=bqk_sb, in_=bqk_d.ap())
            bv_sb = const.tile([P, 2], f32, name="bv_sb")
            nc.sync.dma_start(out=bv_sb, in_=bv_d.ap())
            eps_sb = const.tile([P, 1], f32, name="eps_sb")
            nc.vector.memset(eps_sb, EPS)

            dram = top.enter_context(tc.tile_pool(name="dram", bufs=1, space="DRAM"))

            with ExitStack() as phase1:
                # phase-1 activations (freed after the AllToAll send)
                p1 = phase1.enter_context(tc.tile_pool(name="p1", bufs=1))
                qk_sb = [p1.tile([P, N], f32r, name=f"qk{m}") for m in range(4)]
                v_sb = [p1.tile([P, 4, 65], f32r, name=f"v{m}") for m in range(MC16)]
                o_sb = [p1.tile([P, N], f32r, name=f"o{pr}") for pr in range(2)]
                rstd_bc = p1.tile([P, N], f32, name="rstd_bc")
                rstd_col = p1.tile([P, MC16], f32, name="rstd_col")

                # ---------------- LN1 stats + qkv ----------------
                with ExitStack() as ph1:
                    xt_pool = ph1.enter_context(tc.tile_pool(name="xt", bufs=1))
                    w_pool = ph1.enter_context(tc.tile_pool(name="wqkv", bufs=1))
                    st_pool = ph1.enter_context(tc.tile_pool(name="st", bufs=2))

                    xt = []
                    for c in range(C8):
                        t = xt_pool.tile([P, N], f32r, name=f"xt{c}")
                        nc.sync.dma_start(out=t, in_=xT_d.ap()[c * P:(c + 1) * P, :])
                        xt.append(t)
                    wqk = []
                    for c in range(C8):
                        t = w_pool.tile([P, 512], f32r, name=f"wqk{c}")
                        nc.sync.dma_start(out=t, in_=wqk_d.ap()[c * P:(c + 1) * P, :])
                        wqk.append(t)
                    wv = []
                    for c in range(C8):
                        t = w_pool.tile([P, 256], f32r, name=f"wv{c}")
                        nc.sync.dma_start(out=t, in_=wv_d.ap()[c * P:(c + 1) * P, :])
                        wv.append(t)

                    # stats: S1 = sum_c x, S2 = sum_c x^2 (column sums via ones-matmul)
                    with ExitStack() as stc:
                        sps = stc.enter_context(
                            tc.tile_pool(name="sps", bufs=1, space="PSUM"))
                        s1p = [sps.tile([32, 512], f32, name=f"s1p{nt}", tag=f"s1p{nt}")
                               for nt in range(NT)]
                        s2p = [sps.tile([32, 512], f32, name=f"s2p{nt}", tag=f"s2p{nt}")
                               for nt in range(NT)]
                        for c in range(C8):
                            sq_t = st_pool.tile([P, N], f32r, name="sq", tag="sq")
                            nc.scalar.activation(out=sq_t, in_=xt[c].bitcast(f32),
                                                 func=AF.Square)
                            for nt in range(NT):
                                nsl = slice(nt * 512, (nt + 1) * 512)
                                nc.tensor.matmul(s1p[nt], ones_r, xt[c][:, nsl],
                                                 start=(c == 0), stop=(c == C8 - 1),
                                                 skip_group_check=True)
                                nc.tensor.matmul(s2p[nt], ones_r, sq_t[:, nsl],
                                                 start=(c == 0), stop=(c == C8 - 1),
                                                 skip_group_check=True)
                        # engines can only address partition bases 0/32/64/96,
                        # so stage the 4 psum rows on one partition and bounce
                        # through DRAM to get a [4, 512] layout
                        st1_row = p1.tile([1, N], f32, name="st1_row")
                        st2_row = p1.tile([1, N], f32, name="st2_row")
                        for nt in range(NT):
                            nsl = slice(nt * 512, (nt + 1) * 512)
                            nc.vector.tensor_copy(st1_row[:, nsl], s1p[nt][0:1, :])
                            nc.vector.tensor_copy(st2_row[:, nsl], s2p[nt][0:1, :])
                        st_d = dram.tile([2, 4, 512], f32, name="st_d")
                        nc.sync.dma_start(
                            out=st_d[0].rearrange("a b -> (a b)"), in_=st1_row)
                        nc.sync.dma_start(
                            out=st_d[1].rearrange("a b -> (a b)"), in_=st2_row)
                        stats1 = p1.tile([4, 512], f32, name="stats1")
                        stats2 = p1.tile([4, 512], f32, name="stats2")
                        nc.sync.dma_start(out=stats1, in_=st_d[0])
                        nc.sync.dma_start(out=stats2, in_=st_d[1])

                    # rstd = 1/sqrt(S2/C - (S1/C)^2 + eps)   on [4, 512]
                    mu4 = p1.tile([4, 512], f32, name="mu4")
                    var4 = p1.tile([4, 512], f32, name="var4")
                    nc.vector.tensor_scalar_mul(mu4, stats1, 1.0 / DIM)
                    nc.vector.tensor_scalar_mul(var4, stats2, 1.0 / DIM)
                    msq = p1.tile([4, 512], f32, name="msq")
                    nc.vector.tensor_mul(msq, mu4, mu4)
                    nc.vector.tensor_sub(var4, var4, msq)
                    sd4 = p1.tile([4, 512], f32, name="sd4")
                    nc.scalar.activation(out=sd4, in_=var4, func=AF.Sqrt,
                                         bias=eps_sb[0:4, :])
                    rstd4 = p1.tile([4, 512], f32, name="rstd4")
                    nc.vector.reciprocal(rstd4, sd4)
                    # broadcast rstd along tokens-on-free (rstd_bc) and
                    # tokens-on-partitions (rstd_col) via a DRAM bounce
                    # (partition-broadcast APs are only legal on DRAM tensors)
                    rs_d = dram.tile([MC16, P], f32, name="rs_d")
                    nc.sync.dma_start(
                        out=rs_d.rearrange("(a b) c -> a (b c)", b=4), in_=rstd4)
                    rs_flat = rs_d.rearrange("a b -> (a b)")
                    for nt in range(NT):
                        nc.sync.dma_start(
                            out=rstd_bc[:, nt * 512:(nt + 1) * 512],
                            in_=bc_row(rs_flat[nt * 512:(nt + 1) * 512], P))
                    nc.sync.dma_start(
                        out=rstd_col, in_=rs_d.rearrange("c p -> p c"))

                    # q,k feature-major: qk_sb[mi] = (W0_qk^T x)[mi]*rstd + b_qk[mi]
                    with ExitStack() as qkc:
                        qps = qkc.enter_context(
                            tc.tile_pool(name="qps", bufs=3, space="PSUM"))
                        for mi in range(4):
                            for nt in range(NT):
                                nsl = slice(nt * 512, (nt + 1) * 512)
                                ps = qps.tile([P, 512], f32, name="qkp", tag="qkp")
                                for c in range(C8):
                                    nc.tensor.matmul(
                                        ps, wqk[c][:, mi * P:(mi + 1) * P],
                                        xt[c][:, nsl],
                                        start=(c == 0), stop=(c == C8 - 1))
                                nc.vector.tensor_tensor(
                                    out=qk_sb[mi][:, nsl],
                                    in0=ps, in1=rstd_bc[:, nsl], op=ALU.mult)
                            nc.vector.tensor_scalar_add(
                                qk_sb[mi], qk_sb[mi].bitcast(f32),
                                bqk_sb[:, mi:mi + 1])
                        # v token-major: v_sb[mc] = (x^T W0_v)[mc] * rstd_col[mc]
                        for mc in range(MC16):
                            ps = qps.tile([P, 256], f32, name="vp", tag="vp", bufs=2)
                            for c in range(C8):
                                nc.tensor.matmul(ps, xt[c][:, mc * P:(mc + 1) * P],
                                                 wv[c],
                                                 start=(c == 0), stop=(c == C8 - 1))
                            nc.vector.tensor_scalar_mul(
                                v_sb[mc][:, :, 0:64],
                                ps.rearrange("p (h d) -> p h d", h=4),
                                rstd_col[:, mc:mc + 1])
                            nc.vector.memset(v_sb[mc].bitcast(f32)[:, :, 64:65], 1.0)

                # ---------------- attention ----------------
                with ExitStack() as ph2:
                    e_pool = ph2.enter_context(tc.tile_pool(name="epool", bufs=2))
                    r_pool = ph2.enter_context(tc.tile_pool(name="rpool", bufs=2))
                    aps = ph2.enter_context(
                        tc.tile_pool(name="aps", bufs=1, space="PSUM"))
                    for nt in range(NT):
                        nsl = slice(nt * 512, (nt + 1) * 512)
                        o_ps = [aps.tile([65, 512], f32, name=f"ops{h}",
                                         tag=f"ops{h}") for h in range(4)]
                        for mc in range(MC16):
                            msl = slice(mc * P, (mc + 1) * P)
                            for h in range(4):
                                pr, hh = divmod(h, 2)
                                qt, kt = qk_sb[pr], qk_sb[2 + pr]
                                hsl = slice(hh * 64, (hh + 1) * 64)
                                s_ps = aps.tile([P, 512], f32, name=f"sps{h % 2}",
                                                tag=f"sps{h % 2}")
                                nc.tensor.matmul(s_ps, kt[hsl, msl], qt[hsl, nsl],
                                                 start=True, stop=True)
                                e_t = e_pool.tile([P, 512], f32r, name=f"e{h}",
                                                  tag=f"e{h}")
                                nc.scalar.activation(out=e_t, in_=s_ps, func=AF.Exp,
                                                     scale=float(HD) ** -0.5)
                                # [V_h | 1] x E_h: rows 0:64 = o_h, row 64 = denom
                                nc.tensor.matmul(
                                    o_ps[h], v_sb[mc][:, h, :], e_t,
                                    start=(mc == 0), stop=(mc == MC16 - 1),
                                    skip_group_check=True)
                        rv_d = dram.tile([4, 512], f32, name="rv_d", tag="rv_d",
                                         bufs=2)
                        for h in range(4):
                            rinv = r_pool.tile([1, 512], f32, name="rinv",
                                               tag="rinv")
                            nc.vector.reciprocal(rinv, o_ps[h][64:65, :])
                            nc.sync.dma_start(out=rv_d[h:h + 1, :], in_=rinv)
                        for h in range(4):
                            pr, hh = divmod(h, 2)
                            rbc = r_pool.tile([64, 512], f32, name=f"rbc{h % 2}",
                                              tag=f"rbc{h % 2}")
                            nc.sync.dma_start(
                                out=rbc, in_=bc_row(rv_d[h:h + 1, :], 64))
                            nc.vector.tensor_tensor(
                                out=o_sb[pr][hh * 64:(hh + 1) * 64, nsl],
                                in0=o_ps[h][0:64, :], in1=rbc, op=ALU.mult)
                    for pr in range(2):
                        nc.vector.tensor_scalar_add(
                            o_sb[pr], o_sb[pr].bitcast(f32), bv_sb[:, pr:pr + 1])

                # ---------------- AllToAll send ----------------
                # 8-core AllToAll: block j carries my 4 heads for the 256-token
                # slice of MY batch that core j will own in phase 2.
                send = dram.tile([8, 256, 256], f32r, name="a2a_send")
                for j in range(8):
                    for pr in range(2):
                        nc.sync.dma_start(
                            out=send[j, pr * P:(pr + 1) * P, :],
                            in_=o_sb[pr][:, j * 256:(j + 1) * 256])

            recv = dram.tile([8, 256, 256], f32r, name="a2a_recv")
            nc.gpsimd.collective_compute(
                "AllToAll", mybir.AluOpType.bypass,
                replica_groups=[list(range(8))],
                ins=[send.opt()], outs=[recv.opt()])

            # ---------------- phase 2: proj + residual + LN2 ----------------
            # phase-2 tokens on this core: 256 of batch 0 (cols 0:256) then
            # 256 of batch 1 (cols 256:512); head-chunk c of the gathered
            # o^T comes from group-rank c//2, pair c%2, per batch.
            p2 = top.enter_context(tc.tile_pool(name="p2", bufs=1))
            of_T = [p2.tile([P, 512], f32r, name=f"of{i}") for i in range(C8)]
            for i in range(C8):
                for g in range(2):
                    nc.sync.dma_start(
                        out=of_T[i][:, g * 256:(g + 1) * 256],
                        in_=recv[g * 4 + i // 2, (i % 2) * P:(i % 2 + 1) * P, :])
            xpb = [p2.tile([P, DIM], f32, name=f"xpb{n4}") for n4 in range(4)]
            for n4 in range(4):
                nc.sync.dma_start(out=xpb[n4],
                                  in_=xpb_d.ap()[n4 * P:(n4 + 1) * P, :])
            x2_sb = [p2.tile([P, DIM], f32, name=f"x2{n4}") for n4 in range(4)]
            h2T = [p2.tile([P, TOKS], f32r, name=f"h2T{c}") for c in range(C8)]

            with ExitStack() as ph4:
                wp_pool = ph4.enter_context(tc.tile_pool(name="wp", bufs=4))
                h2_pool = ph4.enter_context(tc.tile_pool(name="h2p", bufs=2))
                pps = ph4.enter_context(tc.tile_pool(name="pps", bufs=1, space="PSUM"))
                tps = ph4.enter_context(tc.tile_pool(name="tps", bufs=2, space="PSUM"))
                for ct in range(2):
                    csl = slice(ct * 512, (ct + 1) * 512)
                    pj_ps = [pps.tile([P, 512], f32, name=f"pj{n4}", tag=f"pj{n4}")
                             for n4 in range(4)]
                    for hdc in range(C8):
                        wpt = wp_pool.tile([P, 512], f32r, name="wpt", tag="wpt")
                        nc.sync.dma_start(
                            out=wpt, in_=wp_d.ap()[hdc * P:(hdc + 1) * P, csl])
                        for n4 in range(4):
                            nc.tensor.matmul(pj_ps[n4],
                                             of_T[hdc][:, n4 * P:(n4 + 1) * P], wpt,
                                             start=(hdc == 0), stop=(hdc == C8 - 1))
                    for n4 in range(4):
                        nc.vector.tensor_tensor(out=x2_sb[n4][:, csl],
                                                in0=pj_ps[n4], in1=xpb[n4][:, csl],
                                                op=ALU.add)
                # LN2 (token-major, bn_stats) + transpose to h2T
                for n4 in range(4):
                    st = h2_pool.tile([P, 2, 6], f32, name="bnst", tag="bnst")
                    for g in range(2):
                        nc.vector.bn_stats(out=st[:, g, :],
                                           in_=x2_sb[n4][:, g * 512:(g + 1) * 512])
                    mv = h2_pool.tile([P, 2], f32, name="bnmv", tag="bnmv")
                    nc.vector.bn_aggr(out=mv, in_=st)
                    sd = h2_pool.tile([P, 1], f32, name="sd2", tag="sd2")
                    nc.scalar.activation(out=sd, in_=mv[:, 1:2], func=AF.Sqrt,
                                         bias=eps_sb)
                    rstd2 = h2_pool.tile([P, 1], f32, name="rstd2", tag="rstd2")
                    nc.vector.reciprocal(rstd2, sd)
                    h2t_ = h2_pool.tile([P, DIM], f32, name="h2t_", tag="h2t_")
                    nc.vector.tensor_scalar(out=h2t_, in0=x2_sb[n4],
                                            scalar1=mv[:, 0:1], scalar2=rstd2,
                                            op0=ALU.subtract, op1=ALU.mult)
                    for c in range(C8):
                        tp = tps.tile([P, P], f32, name="tp", tag="tp")
                        nc.tensor.transpose(tp, h2t_[:, c * P:(c + 1) * P], ident)
                        nc.vector.tensor_copy(h2T[c][:, n4 * P:(n4 + 1) * P], tp)

            # ---------------- MLP ----------------
            bf1_sb = p2.tile([P, FF32], f32, name="bf1_sb")
            nc.sync.dma_start(out=bf1_sb, in_=bf1_d.ap())
            g_T = [p2.tile([P, TOKS], f32r, name=f"gT{f}") for f in range(FF32)]
            yout = [p2.tile([P, DIM], f32, name=f"yo{n4}") for n4 in range(4)]
            with ExitStack() as ph5:
                w1_pool = ph5.enter_context(tc.tile_pool(name="w1", bufs=6))
                f1ps = ph5.enter_context(tc.tile_pool(name="f1ps", bufs=1,
                                                      space="PSUM"))
                for ffg in range(8):
                    f1_ps = [f1ps.tile([P, 512], f32, name=f"f1p{f}", tag=f"f1p{f}")
                             for f in range(4)]
                    for c in range(C8):
                        w1t = w1_pool.tile([P, 512], f32r, name="w1t", tag="w1t")
                        nc.sync.dma_start(
                            out=w1t, in_=wf1_d.ap()[c * P:(c + 1) * P,
                                                    ffg * 512:(ffg + 1) * 512])
                        for f in range(4):
                            nc.tensor.matmul(f1_ps[f], w1t[:, f * P:(f + 1) * P],
                                             h2T[c],
                                             start=(c == 0), stop=(c == C8 - 1))
                    for f in range(4):
                        ffc = ffg * 4 + f
                        nc.scalar.activation(out=g_T[ffc], in_=f1_ps[f],
                                             func=AF.Gelu,
                                             bias=bf1_sb[:, ffc:ffc + 1])
            with ExitStack() as ph6:
                w2_pool = ph6.enter_context(tc.tile_pool(name="w2", bufs=6))
                f2ps = ph6.enter_context(tc.tile_pool(name="f2ps", bufs=1,
                                                      space="PSUM"))
                for ct in range(2):
                    csl = slice(ct * 512, (ct + 1) * 512)
                    f2_ps = [f2ps.tile([P, 512], f32, name=f"f2p{n4}", tag=f"f2p{n4}")
                             for n4 in range(4)]
                    for ffc in range(FF32):
                        w2t = w2_pool.tile([P, 512], f32r, name="w2t", tag="w2t")
                        nc.sync.dma_start(
                            out=w2t, in_=wf2_d.ap()[ffc * P:(ffc + 1) * P, csl])
                        for n4 in range(4):
                            nc.tensor.matmul(f2_ps[n4],
                                             g_T[ffc][:, n4 * P:(n4 + 1) * P], w2t,
                                             start=(ffc == 0), stop=(ffc == FF32 - 1))
                    for n4 in range(4):
                        nc.vector.tensor_tensor(out=yout[n4][:, csl],
                                                in0=f2_ps[n4], in1=x2_sb[n4][:, csl],
                                                op=ALU.add)
            for n4 in range(4):
                nc.sync.dma_start(out=yout_d.ap()[n4 * P:(n4 + 1) * P, :],
                                  in_=yout[n4])

    nc.compile()
    _CACHE["nc"] = nc
    return nc


def _prep_inputs(x, ln1_g, ln1_b, qkv_w, proj_w, proj_b, ln2_g, ln2_b,
                 fc1_w, fc1_b, fc2_w, fc2_b):
    """Host-side sharding + weight folding. Returns list of 8 in_maps."""
    f4 = np.float32
    x = np.asarray(x, f4)
    qkv_w = np.asarray(qkv_w, f4)
    w1 = qkv_w * np.asarray(ln1_g, f4)[:, None]          # gamma fold
    w1c = w1 - w1.mean(axis=0, keepdims=True)            # mean-centering fold
    bqkv = np.asarray(ln1_b, f4) @ qkv_w                 # beta fold -> bias [3*DIM]
    w1c_r = w1c.reshape(DIM, 3, HEADS, HD)
    bqkv_r = bqkv.reshape(3, HEADS, HD)

    wf1 = np.asarray(fc1_w, f4) * np.asarray(ln2_g, f4)[:, None]
    wf1c = np.ascontiguousarray(wf1 - wf1.mean(axis=0, keepdims=True))
    bf1 = np.asarray(fc1_b, f4) + np.asarray(ln2_b, f4) @ np.asarray(fc1_w, f4)

    in_maps = []
    for core in range(NCORES):
        g, j = divmod(core, 4)
        h0 = 4 * j
        wq = w1c_r[:, 0, h0:h0 + 4, :].reshape(DIM, 256)
        wk = w1c_r[:, 1, h0:h0 + 4, :].reshape(DIM, 256)
        wv = w1c_r[:, 2, h0:h0 + 4, :].reshape(DIM, 256)
        bq = bqkv_r[0, h0:h0 + 4, :].reshape(256)
        bk = bqkv_r[1, h0:h0 + 4, :].reshape(256)
        bv = bqkv_r[2, h0:h0 + 4, :].reshape(256)
        in_maps.append({
            "xT": np.ascontiguousarray(x[g].T),
            "w_qk": np.ascontiguousarray(np.concatenate([wq, wk], axis=1)),
            "w_v": np.ascontiguousarray(wv),
            "b_qk": np.ascontiguousarray(
                np.concatenate([bq, bk]).reshape(4, P).T),
            "b_v": np.ascontiguousarray(bv.reshape(2, P).T),
            "w_p": np.asarray(proj_w, f4),
            "x_pb": np.ascontiguousarray(
                np.concatenate([x[0, core * 256:(core + 1) * 256, :],
                                x[1, core * 256:(core + 1) * 256, :]], axis=0)
                + np.asarray(proj_b, f4)),
            "w_f1": wf1c,
            "b_f1": np.ascontiguousarray(bf1.reshape(FF32, P).T),
            "w_f2": np.asarray(fc2_w, f4),
            "ones_c": np.ones((P, 32), f4),
        })
    return in_maps


def kernel(**inputs):
    from concourse.bass_utils import run_bass_kernel_spmd

    nc = _build()
    in_maps = _prep_inputs(**inputs)
    res = run_bass_kernel_spmd(nc, in_maps, core_ids=list(range(NCORES)))
    return assemble_output(res.results, inputs)


def assemble_output(results, inputs):
    fc2_b = np.asarray(inputs["fc2_b"], np.float32)
    out = np.empty((B, N, DIM), np.float32)
    for core in range(NCORES):
        y = results[core]["y_out"] + fc2_b
        out[0, core * 256:(core + 1) * 256, :] = y[0:256]
        out[1, core * 256:(core + 1) * 256, :] = y[256:512]
    return out


# revision 19
# speedup vs baseline: 1.0022x; 1.0022x over previous
"""Trainium2 Bass kernel for a pre-norm transformer block (B=2, N=2048, D=1024, H=16, FF=4096).

Strategy (8 cores):
  Phase 1 (DP on batch x TP on heads): cores 0-3 handle batch 0, cores 4-7 batch 1;
    each core computes LN1 + qkv + attention for its 4 heads over all 2048 tokens,
    in feature-major ("transposed") layout so no on-chip transposes are needed.
    LN1 is folded into the weights host-side (gamma-scaling + mean-centering of the
    qkv weight columns); the per-token rstd is applied after the matmul via a
    DMA-broadcast row. Softmax runs without max-subtraction (scores are O(10), fp32
    exp is safe); denominators come from column-packed ones-matmuls.
  Reshard: one AllToAll per 4-core group moves head-shards -> token-shards.
  Phase 2 (token-parallel): each core runs proj + residual + LN2 + MLP for its 512
    tokens with full weights. proj_b is pre-added to the residual host-side; ln2_g/b
    are folded into fc1_w/fc1_b host-side; fc2_b is added host-side after gather.

All matmuls run in float32r (full PE rate at N>=256, ~1e-3 matmul accuracy).
"""

import numpy as np

DIM = 1024
HEADS = 16
HD = 64
FF = 4096
B = 2
N = 2048
EPS = 1e-5
P = 128
NCORES = 8
GROUPS = [[0, 1, 2, 3], [4, 5, 6, 7]]
TOKS = 512         # tokens per core in phase 2
C8 = DIM // P      # 8 contraction chunks
NT = N // 512      # 4 n-tiles
MC16 = N // P      # 16 m-chunks
FF32 = FF // P     # 32 ff chunks

_CACHE = {}


def _build():
    if "nc" in _CACHE:
        return _CACHE["nc"]

    import concourse.bacc as bacc
    import concourse.bass as bass
    import concourse.tile as tile
    from concourse import mybir
    from concourse.masks import make_identity
    from contextlib import ExitStack

    f32 = mybir.dt.float32
    f32r = mybir.dt.float32r
    AF = mybir.ActivationFunctionType
    ALU = mybir.AluOpType

    nc = bacc.Bacc("TRN2", target_bir_lowering=False, debug=False,
                   num_devices=NCORES)

    # ---- per-core dram tensors ----
    xT_d = nc.dram_tensor("xT", [DIM, N], f32r, kind="ExternalInput")
    wqk_d = nc.dram_tensor("w_qk", [DIM, 512], f32r, kind="ExternalInput")
    wv_d = nc.dram_tensor("w_v", [DIM, 256], f32r, kind="ExternalInput")
    bqk_d = nc.dram_tensor("b_qk", [P, 4], f32, kind="ExternalInput")
    bv_d = nc.dram_tensor("b_v", [P, 2], f32, kind="ExternalInput")
    wp_d = nc.dram_tensor("w_p", [DIM, DIM], f32r, kind="ExternalInput")
    xpb_d = nc.dram_tensor("x_pb", [TOKS, DIM], f32, kind="ExternalInput")
    wf1_d = nc.dram_tensor("w_f1", [DIM, FF], f32r, kind="ExternalInput")
    bf1_d = nc.dram_tensor("b_f1", [P, FF32], f32, kind="ExternalInput")
    wf2_d = nc.dram_tensor("w_f2", [FF, DIM], f32r, kind="ExternalInput")
    ones_d = nc.dram_tensor("ones_c", [P, 32], f32r, kind="ExternalInput")
    yout_d = nc.dram_tensor("y_out", [TOKS, DIM], f32, kind="ExternalOutput")

    def bc_row(ap_row, parts):
        # partition-broadcast AP for DMA: read one row into `parts` partitions
        t = ap_row
        dims = [list(d) for d in t.ap]
        if dims and dims[0][1] == 1:
            dims = dims[1:]
        return bass.AP(tensor=t.tensor, offset=t.offset,
                       ap=[[0, parts]] + dims)

    with tile.TileContext(nc) as tc:
        with ExitStack() as top:
            const = top.enter_context(tc.tile_pool(name="const", bufs=1))
            ones_r = const.tile([P, 32], f32r, name="ones_r")
            nc.sync.dma_start(out=ones_r, in_=ones_d.ap())
            ident = const.tile([P, P], f32, name="ident")
            make_identity(nc, ident)
            bqk_sb = const.tile([P, 4], f32, name="bqk_sb")
            nc.sync.dma_start(out=bqk_sb, in_=bqk_d.ap())
            bv_sb = const.tile([P, 2], f32, name="bv_sb")
            nc.sync.dma_start(out=bv_sb, in_=bv_d.ap())
            eps_sb = const.tile([P, 1], f32, name="eps_sb")
            nc.vector.memset(eps_sb, EPS)

            dram = top.enter_context(tc.tile_pool(name="dram", bufs=1, space="DRAM"))

            with ExitStack() as phase1:
                # phase-1 activations (freed after the AllToAll send)
                p1 = phase1.enter_context(tc.tile_pool(name="p1", bufs=1))
                qk_sb = [p1.tile([P, N], f32r, name=f"qk{m}") for m in range(4)]
                v_sb = [p1.tile([P, 4, 65], f32r, name=f"v{m}") for m in range(MC16)]
                o_sb = [p1.tile([P, N], f32r, name=f"o{pr}") for pr in range(2)]
                rstd_bc = p1.tile([P, N], f32, name="rstd_bc")
                rstd_col = p1.tile([P, MC16], f32, name="rstd_col")

                # ---------------- LN1 stats + qkv ----------------
                with ExitStack() as ph1:
                    xt_pool = ph1.enter_context(tc.tile_pool(name="xt", bufs=1))
                    w_pool = ph1.enter_context(tc.tile_pool(name="wqkv", bufs=1))
                    st_pool = ph1.enter_context(tc.tile_pool(name="st", bufs=2))

                    xt = []
                    for c in range(C8):
                        t = xt_pool.tile([P, N], f32r, name=f"xt{c}")
                        nc.sync.dma_start(out=t, in_=xT_d.ap()[c * P:(c + 1) * P, :])
                        xt.append(t)
                    wqk = []
                    for c in range(C8):
                        t = w_pool.tile([P, 512], f32r, name=f"wqk{c}")
                        nc.sync.dma_start(out=t, in_=wqk_d.ap()[c * P:(c + 1) * P, :])
                        wqk.append(t)
                    wv = []
                    for c in range(C8):
                        t = w_pool.tile([P, 256], f32r, name=f"wv{c}")
                        nc.sync.dma_start(out=t, in_=wv_d.ap()[c * P:(c + 1) * P, :])
                        wv.append(t)

                    # stats: S1 = sum_c x, S2 = sum_c x^2 (column sums via ones-matmul)
                    with ExitStack() as stc:
                        sps = stc.enter_context(
                            tc.tile_pool(name="sps", bufs=1, space="PSUM"))
                        s1p = [sps.tile([32, 512], f32, name=f"s1p{nt}", tag=f"s1p{nt}")
                               for nt in range(NT)]
                        s2p = [sps.tile([32, 512], f32, name=f"s2p{nt}", tag=f"s2p{nt}")
                               for nt in range(NT)]
                        for c in range(C8):
                            sq_t = st_pool.tile([P, N], f32r, name="sq", tag="sq")
                            nc.scalar.activation(out=sq_t, in_=xt[c].bitcast(f32),
                                                 func=AF.Square)
                            for nt in range(NT):
                                nsl = slice(nt * 512, (nt + 1) * 512)
                                nc.tensor.matmul(s1p[nt], ones_r, xt[c][:, nsl],
                                                 start=(c == 0), stop=(c == C8 - 1),
                                                 skip_group_check=True)
                                nc.tensor.matmul(s2p[nt], ones_r, sq_t[:, nsl],
                                                 start=(c == 0), stop=(c == C8 - 1),
                                                 skip_group_check=True)
                        # engines can only address partition bases 0/32/64/96,
                        # so stage the 4 psum rows on one partition and bounce
                        # through DRAM to get a [4, 512] layout
                        st1_row = p1.tile([1, N], f32, name="st1_row")
                        st2_row = p1.tile([1, N], f32, name="st2_row")
                        for nt in range(NT):
                            nsl = slice(nt * 512, (nt + 1) * 512)
                            nc.vector.tensor_copy(st1_row[:, nsl], s1p[nt][0:1, :])
                            nc.vector.tensor_copy(st2_row[:, nsl], s2p[nt][0:1, :])
                        st_d = dram.tile([2, 4, 512], f32, name="st_d")
                        nc.sync.dma_start(
                            out=st_d[0].rearrange("a b -> (a b)"), in_=st1_row)
                        nc.sync.dma_start(
                            out=st_d[1].rearrange("a b -> (a b)"), in_=st2_row)
                        stats1 = p1.tile([4, 512], f32, name="stats1")
                        stats2 = p1.tile([4, 512], f32, name="stats2")
                        nc.sync.dma_start(out=stats1, in_=st_d[0])
                        nc.sync.dma_start(out=stats2, in_=st_d[1])

                    # rstd = 1/sqrt(S2/C - (S1/C)^2 + eps)   on [4, 512]
                    mu4 = p1.tile([4, 512], f32, name="mu4")
                    var4 = p1.tile([4, 512], f32, name="var4")
                    nc.vector.tensor_scalar_mul(mu4, stats1, 1.0 / DIM)
                    nc.vector.tensor_scalar_mul(var4, stats2, 1.0 / DIM)
                    msq = p1.tile([4, 512], f32, name="msq")
                    nc.vector.tensor_mul(msq, mu4, mu4)
                    nc.vector.tensor_sub(var4, var4, msq)
                    sd4 = p1.tile([4, 512], f32, name="sd4")
                    nc.scalar.activation(out=sd4, in_=var4, func=AF.Sqrt,
                                         bias=eps_sb[0:4, :])
                    rstd4 = p1.tile([4, 512], f32, name="rstd4")
                    nc.vector.reciprocal(rstd4, sd4)
                    # broadcast rstd along tokens-on-free (rstd_bc) and
                    # tokens-on-partitions (rstd_col) via a DRAM bounce
                    # (partition-broadcast APs are only legal on DRAM tensors)
                    rs_d = dram.tile([MC16, P], f32, name="rs_d")
                    nc.sync.dma_start(
                        out=rs_d.rearrange("(a b) c -> a (b c)", b=4), in_=rstd4)
                    rs_flat = rs_d.rearrange("a b -> (a b)")
                    for nt in range(NT):
                        nc.sync.dma_start(
                            out=rstd_bc[:, nt * 512:(nt + 1) * 512],
                            in_=bc_row(rs_flat[nt * 512:(nt + 1) * 512], P))
                    nc.sync.dma_start(
                        out=rstd_col, in_=rs_d.rearrange("c p -> p c"))

                    # q,k feature-major: qk_sb[mi] = (W0_qk^T x)[mi]*rstd + b_qk[mi]
                    with ExitStack() as qkc:
                        qps = qkc.enter_context(
                            tc.tile_pool(name="qps", bufs=3, space="PSUM"))
                        for mi in range(4):
                            for nt in range(NT):
                                nsl = slice(nt * 512, (nt + 1) * 512)
                                ps = qps.tile([P, 512], f32, name="qkp", tag="qkp")
                                for c in range(C8):
                                    nc.tensor.matmul(
                                        ps, wqk[c][:, mi * P:(mi + 1) * P],
                                        xt[c][:, nsl],
                                        start=(c == 0), stop=(c == C8 - 1))
                                nc.vector.tensor_tensor(
                                    out=qk_sb[mi][:, nsl],
                                    in0=ps, in1=rstd_bc[:, nsl], op=ALU.mult)
                            nc.vector.tensor_scalar_add(
                                qk_sb[mi], qk_sb[mi].bitcast(f32),
                                bqk_sb[:, mi:mi + 1])
                        # v token-major: v_sb[mc] = (x^T W0_v)[mc] * rstd_col[mc]
                        for mc in range(MC16):
                            ps = qps.tile([P, 256], f32, name="vp", tag="vp", bufs=2)
                            for c in range(C8):
                                nc.tensor.matmul(ps, xt[c][:, mc * P:(mc + 1) * P],
                                                 wv[c],
                                                 start=(c == 0), stop=(c == C8 - 1))
                            nc.vector.tensor_scalar_mul(
                                v_sb[mc][:, :, 0:64],
                                ps.rearrange("p (h d) -> p h d", h=4),
                                rstd_col[:, mc:mc + 1])
                            nc.vector.memset(v_sb[mc].bitcast(f32)[:, :, 64:65], 1.0)

                # ---------------- attention ----------------
                with ExitStack() as ph2:
                    e_pool = ph2.enter_context(tc.tile_pool(name="epool", bufs=2))
                    r_pool = ph2.enter_context(tc.tile_pool(name="rpool", bufs=2))
                    aps = ph2.enter_context(
                        tc.tile_pool(name="aps", bufs=1, space="PSUM"))
                    for nt in range(NT):
                        nsl = slice(nt * 512, (nt + 1) * 512)
                        o_ps = [aps.tile([65, 512], f32, name=f"ops{h}",
                                         tag=f"ops{h}") for h in range(4)]
                        for mc in range(MC16):
                            msl = slice(mc * P, (mc + 1) * P)
                            for h in range(4):
                                pr, hh = divmod(h, 2)
                                qt, kt = qk_sb[pr], qk_sb[2 + pr]
                                hsl = slice(hh * 64, (hh + 1) * 64)
                                s_ps = aps.tile([P, 512], f32, name=f"sps{h}",
                                                tag=f"sps{h}")
                                nc.tensor.matmul(s_ps, kt[hsl, msl], qt[hsl, nsl],
                                                 start=True, stop=True)
                                e_t = e_pool.tile([P, 512], f32r, name=f"e{h}",
                                                  tag=f"e{h}")
                                nc.scalar.activation(out=e_t, in_=s_ps, func=AF.Exp,
                                                     scale=float(HD) ** -0.5)
                                # [V_h | 1] x E_h: rows 0:64 = o_h, row 64 = denom
                                nc.tensor.matmul(
                                    o_ps[h], v_sb[mc][:, h, :], e_t,
                                    start=(mc == 0), stop=(mc == MC16 - 1),
                                    skip_group_check=True)
                        rv_d = dram.tile([4, 512], f32, name="rv_d", tag="rv_d",
                                         bufs=2)
                        for h in range(4):
                            rinv = r_pool.tile([1, 512], f32, name="rinv",
                                               tag="rinv")
                            nc.vector.reciprocal(rinv, o_ps[h][64:65, :])
                            nc.sync.dma_start(out=rv_d[h:h + 1, :], in_=rinv)
                        for h in range(4):
                            pr, hh = divmod(h, 2)
                            rbc = r_pool.tile([64, 512], f32, name=f"rbc{h % 2}",
                                              tag=f"rbc{h % 2}")
                            nc.sync.dma_start(
                                out=rbc, in_=bc_row(rv_d[h:h + 1, :], 64))
                            nc.vector.tensor_tensor(
                                out=o_sb[pr][hh * 64:(hh + 1) * 64, nsl],
                                in0=o_ps[h][0:64, :], in1=rbc, op=ALU.mult)
                    for pr in range(2):
                        nc.vector.tensor_scalar_add(
                            o_sb[pr], o_sb[pr].bitcast(f32), bv_sb[:, pr:pr + 1])

                # ---------------- AllToAll send ----------------
                # 8-core AllToAll: block j carries my 4 heads for the 256-token
                # slice of MY batch that core j will own in phase 2.
                send = dram.tile([8, 256, 256], f32r, name="a2a_send")
                for j in range(8):
                    for pr in range(2):
                        nc.sync.dma_start(
                            out=send[j, pr * P:(pr + 1) * P, :],
                            in_=o_sb[pr][:, j * 256:(j + 1) * 256])

            recv = dram.tile([8, 256, 256], f32r, name="a2a_recv")
            nc.gpsimd.collective_compute(
                "AllToAll", mybir.AluOpType.bypass,
                replica_groups=[list(range(8))],
                ins=[send.opt()], outs=[recv.opt()])

            # ---------------- phase 2: proj + residual + LN2 ----------------
            # phase-2 tokens on this core: 256 of batch 0 (cols 0:256) then
            # 256 of batch 1 (cols 256:512); head-chunk c of the gathered
            # o^T comes from group-rank c//2, pair c%2, per batch.
            p2 = top.enter_context(tc.tile_pool(name="p2", bufs=1))
            of_T = [p2.tile([P, 512], f32r, name=f"of{i}") for i in range(C8)]
            for i in range(C8):
                for g in range(2):
                    nc.sync.dma_start(
                        out=of_T[i][:, g * 256:(g + 1) * 256],
                        in_=recv[g * 4 + i // 2, (i % 2) * P:(i % 2 + 1) * P, :])
            xpb = [p2.tile([P, DIM], f32, name=f"xpb{n4}") for n4 in range(4)]
            for n4 in range(4):
                nc.sync.dma_start(out=xpb[n4],
                                  in_=xpb_d.ap()[n4 * P:(n4 + 1) * P, :])
            x2_sb = [p2.tile([P, DIM], f32, name=f"x2{n4}") for n4 in range(4)]
            h2T = [p2.tile([P, TOKS], f32r, name=f"h2T{c}") for c in range(C8)]

            with ExitStack() as ph4:
                wp_pool = ph4.enter_context(tc.tile_pool(name="wp", bufs=4))
                h2_pool = ph4.enter_context(tc.tile_pool(name="h2p", bufs=2))
                pps = ph4.enter_context(tc.tile_pool(name="pps", bufs=1, space="PSUM"))
                tps = ph4.enter_context(tc.tile_pool(name="tps", bufs=2, space="PSUM"))
                for ct in range(2):
                    csl = slice(ct * 512, (ct + 1) * 512)
                    pj_ps = [pps.tile([P, 512], f32, name=f"pj{n4}", tag=f"pj{n4}")
                             for n4 in range(4)]
                    for hdc in range(C8):
                        wpt = wp_pool.tile([P, 512], f32r, name="wpt", tag="wpt")
                        nc.sync.dma_start(
                            out=wpt, in_=wp_d.ap()[hdc * P:(hdc + 1) * P, csl])
                        for n4 in range(4):
                            nc.tensor.matmul(pj_ps[n4],
                                             of_T[hdc][:, n4 * P:(n4 + 1) * P], wpt,
                                             start=(hdc == 0), stop=(hdc == C8 - 1))
                    for n4 in range(4):
                        nc.vector.tensor_tensor(out=x2_sb[n4][:, csl],
                                                in0=pj_ps[n4], in1=xpb[n4][:, csl],
                                                op=ALU.add)
                # LN2 (token-major, bn_stats) + transpose to h2T
                for n4 in range(4):
                    st = h2_pool.tile([P, 2, 6], f32, name="bnst", tag="bnst")
                    for g in range(2):
                        nc.vector.bn_stats(out=st[:, g, :],
                                           in_=x2_sb[n4][:, g * 512:(g + 1) * 512])
                    mv = h2_pool.tile([P, 2], f32, name="bnmv", tag="bnmv")
                    nc.vector.bn_aggr(out=mv, in_=st)
                    sd = h2_pool.tile([P, 1], f32, name="sd2", tag="sd2")
                    nc.scalar.activation(out=sd, in_=mv[:, 1:2], func=AF.Sqrt,
                                         bias=eps_sb)
                    rstd2 = h2_pool.tile([P, 1], f32, name="rstd2", tag="rstd2")
                    nc.vector.reciprocal(rstd2, sd)
                    h2t_ = h2_pool.tile([P, DIM], f32, name="h2t_", tag="h2t_")
                    nc.vector.tensor_scalar(out=h2t_, in0=x2_sb[n4],
                                            scalar1=mv[:, 0:1], scalar2=rstd2,
                                            op0=ALU.subtract, op1=ALU.mult)
                    for c in range(C8):
                        tp = tps.tile([P, P], f32, name="tp", tag="tp")
                        nc.tensor.transpose(tp, h2t_[:, c * P:(c + 1) * P], ident)
                        nc.vector.tensor_copy(h2T[c][:, n4 * P:(n4 + 1) * P], tp)

            # ---------------- MLP ----------------
            bf1_sb = p2.tile([P, FF32], f32, name="bf1_sb")
            nc.sync.dma_start(out=bf1_sb, in_=bf1_d.ap())
            g_T = [p2.tile([P, TOKS], f32r, name=f"gT{f}") for f in range(FF32)]
            yout = [p2.tile([P, DIM], f32, name=f"yo{n4}") for n4 in range(4)]
            with ExitStack() as ph5:
                w1_pool = ph5.enter_context(tc.tile_pool(name="w1", bufs=6))
                f1ps = ph5.enter_context(tc.tile_pool(name="f1ps", bufs=1,
                                                      space="PSUM"))
                for ffg in range(8):
                    f1_ps = [f1ps.tile([P, 512], f32, name=f"f1p{f}", tag=f"f1p{f}")
                             for f in range(4)]
                    for c in range(C8):
                        w1t = w1_pool.tile([P, 512], f32r, name="w1t", tag="w1t")
                        nc.sync.dma_start(
                            out=w1t, in_=wf1_d.ap()[c * P:(c + 1) * P,
                                                    ffg * 512:(ffg + 1) * 512])
                        for f in range(4):
                            nc.tensor.matmul(f1_ps[f], w1t[:, f * P:(f + 1) * P],
                                             h2T[c],
                                             start=(c == 0), stop=(c == C8 - 1))
                    for f in range(4):
                        ffc = ffg * 4 + f
                        nc.scalar.activation(out=g_T[ffc], in_=f1_ps[f],
                                             func=AF.Gelu,
                                             bias=bf1_sb[:, ffc:ffc + 1])
            with ExitStack() as ph6:
                w2_pool = ph6.enter_context(tc.tile_pool(name="w2", bufs=6))
                f2ps = ph6.enter_context(tc.tile_pool(name="f2ps", bufs=1,
                                                      space="PSUM"))
                for ct in range(2):
                    csl = slice(ct * 512, (ct + 1) * 512)
                    f2_ps = [f2ps.tile([P, 512], f32, name=f"f2p{n4}", tag=f"f2p{n4}")
                             for n4 in range(4)]
                    for ffc in range(FF32):
                        w2t = w2_pool.tile([P, 512], f32r, name="w2t", tag="w2t")
                        nc.sync.dma_start(
                            out=w2t, in_=wf2_d.ap()[ffc * P:(ffc + 1) * P, csl])
                        for n4 in range(4):
                            nc.tensor.matmul(f2_ps[n4],
                                             g_T[ffc][:, n4 * P:(n4 + 1) * P], w2t,
                                             start=(ffc == 0), stop=(ffc == FF32 - 1))
                    for n4 in range(4):
                        nc.vector.tensor_tensor(out=yout[n4][:, csl],
                                                in0=f2_ps[n4], in1=x2_sb[n4][:, csl],
                                                op=ALU.add)
            for n4 in range(4):
                nc.sync.dma_start(out=yout_d.ap()[n4 * P:(n4 + 1) * P, :],
                                  in_=yout[n4])

    nc.compile()
    _CACHE["nc"] = nc
    return nc


def _prep_inputs(x, ln1_g, ln1_b, qkv_w, proj_w, proj_b, ln2_g, ln2_b,
                 fc1_w, fc1_b, fc2_w, fc2_b):
    """Host-side sharding + weight folding. Returns list of 8 in_maps."""
    f4 = np.float32
    x = np.asarray(x, f4)
    qkv_w = np.asarray(qkv_w, f4)
    w1 = qkv_w * np.asarray(ln1_g, f4)[:, None]          # gamma fold
    w1c = w1 - w1.mean(axis=0, keepdims=True)            # mean-centering fold
    bqkv = np.asarray(ln1_b, f4) @ qkv_w                 # beta fold -> bias [3*DIM]
    w1c_r = w1c.reshape(DIM, 3, HEADS, HD)
    bqkv_r = bqkv.reshape(3, HEADS, HD)

    wf1 = np.asarray(fc1_w, f4) * np.asarray(ln2_g, f4)[:, None]
    wf1c = np.ascontiguousarray(wf1 - wf1.mean(axis=0, keepdims=True))
    bf1 = np.asarray(fc1_b, f4) + np.asarray(ln2_b, f4) @ np.asarray(fc1_w, f4)

    in_maps = []
    for core in range(NCORES):
        g, j = divmod(core, 4)
        h0 = 4 * j
        wq = w1c_r[:, 0, h0:h0 + 4, :].reshape(DIM, 256)
        wk = w1c_r[:, 1, h0:h0 + 4, :].reshape(DIM, 256)
        wv = w1c_r[:, 2, h0:h0 + 4, :].reshape(DIM, 256)
        bq = bqkv_r[0, h0:h0 + 4, :].reshape(256)
        bk = bqkv_r[1, h0:h0 + 4, :].reshape(256)
        bv = bqkv_r[2, h0:h0 + 4, :].reshape(256)
        in_maps.append({
            "xT": np.ascontiguousarray(x[g].T),
            "w_qk": np.ascontiguousarray(np.concatenate([wq, wk], axis=1)),
            "w_v": np.ascontiguousarray(wv),
            "b_qk": np.ascontiguousarray(
                np.concatenate([bq, bk]).reshape(4, P).T),
            "b_v": np.ascontiguousarray(bv.reshape(2, P).T),
            "w_p": np.asarray(proj_w, f4),
            "x_pb": np.ascontiguousarray(
                np.concatenate([x[0, core * 256:(core + 1) * 256, :],
                                x[1, core * 256:(core + 1) * 256, :]], axis=0)
                + np.asarray(proj_b, f4)),
            "w_f1": wf1c,
            "b_f1": np.ascontiguousarray(bf1.reshape(FF32, P).T),
            "w_f2": np.asarray(fc2_w, f4),
            "ones_c": np.ones((P, 32), f4),
        })
    return in_maps


def kernel(**inputs):
    from concourse.bass_utils import run_bass_kernel_spmd

    nc = _build()
    in_maps = _prep_inputs(**inputs)
    res = run_bass_kernel_spmd(nc, in_maps, core_ids=list(range(NCORES)))
    return assemble_output(res.results, inputs)


def assemble_output(results, inputs):
    fc2_b = np.asarray(inputs["fc2_b"], np.float32)
    out = np.empty((B, N, DIM), np.float32)
    for core in range(NCORES):
        y = results[core]["y_out"] + fc2_b
        out[0, core * 256:(core + 1) * 256, :] = y[0:256]
        out[1, core * 256:(core + 1) * 256, :] = y[256:512]
    return out


# revision 20
# speedup vs baseline: 1.0062x; 1.0039x over previous
"""Trainium2 Bass kernel for a pre-norm transformer block (B=2, N=2048, D=1024, H=16, FF=4096).

Strategy (8 cores):
  Phase 1 (DP on batch x TP on heads): cores 0-3 handle batch 0, cores 4-7 batch 1;
    each core computes LN1 + qkv + attention for its 4 heads over all 2048 tokens,
    in feature-major ("transposed") layout so no on-chip transposes are needed.
    LN1 is folded into the weights host-side (gamma-scaling + mean-centering of the
    qkv weight columns); the per-token rstd is applied after the matmul via a
    DMA-broadcast row. Softmax runs without max-subtraction (scores are O(10), fp32
    exp is safe); denominators come from column-packed ones-matmuls.
  Reshard: one AllToAll per 4-core group moves head-shards -> token-shards.
  Phase 2 (token-parallel): each core runs proj + residual + LN2 + MLP for its 512
    tokens with full weights. proj_b is pre-added to the residual host-side; ln2_g/b
    are folded into fc1_w/fc1_b host-side; fc2_b is added host-side after gather.

All matmuls run in float32r (full PE rate at N>=256, ~1e-3 matmul accuracy).
"""

import numpy as np

DIM = 1024
HEADS = 16
HD = 64
FF = 4096
B = 2
N = 2048
EPS = 1e-5
P = 128
NCORES = 8
GROUPS = [[0, 1, 2, 3], [4, 5, 6, 7]]
TOKS = 512         # tokens per core in phase 2
C8 = DIM // P      # 8 contraction chunks
NT = N // 512      # 4 n-tiles
MC16 = N // P      # 16 m-chunks
FF32 = FF // P     # 32 ff chunks

_CACHE = {}


def _build():
    if "nc" in _CACHE:
        return _CACHE["nc"]

    import concourse.bacc as bacc
    import concourse.bass as bass
    import concourse.tile as tile
    from concourse import mybir
    from concourse.masks import make_identity
    from contextlib import ExitStack

    f32 = mybir.dt.float32
    f32r = mybir.dt.float32r
    AF = mybir.ActivationFunctionType
    ALU = mybir.AluOpType

    nc = bacc.Bacc("TRN2", target_bir_lowering=False, debug=False,
                   num_devices=NCORES)

    # ---- per-core dram tensors ----
    xT_d = nc.dram_tensor("xT", [DIM, N], f32r, kind="ExternalInput")
    wqk_d = nc.dram_tensor("w_qk", [DIM, 512], f32r, kind="ExternalInput")
    wv_d = nc.dram_tensor("w_v", [DIM, 256], f32r, kind="ExternalInput")
    bqk_d = nc.dram_tensor("b_qk", [P, 4], f32, kind="ExternalInput")
    bv_d = nc.dram_tensor("b_v", [P, 2], f32, kind="ExternalInput")
    wp_d = nc.dram_tensor("w_p", [DIM, DIM], f32r, kind="ExternalInput")
    xpb_d = nc.dram_tensor("x_pb", [TOKS, DIM], f32, kind="ExternalInput")
    wf1_d = nc.dram_tensor("w_f1", [DIM, FF], f32r, kind="ExternalInput")
    bf1_d = nc.dram_tensor("b_f1", [P, FF32], f32, kind="ExternalInput")
    wf2_d = nc.dram_tensor("w_f2", [FF, DIM], f32r, kind="ExternalInput")
    ones_d = nc.dram_tensor("ones_c", [P, 32], f32r, kind="ExternalInput")
    yout_d = nc.dram_tensor("y_out", [TOKS, DIM], f32, kind="ExternalOutput")

    def bc_row(ap_row, parts):
        # partition-broadcast AP for DMA: read one row into `parts` partitions
        t = ap_row
        dims = [list(d) for d in t.ap]
        if dims and dims[0][1] == 1:
            dims = dims[1:]
        return bass.AP(tensor=t.tensor, offset=t.offset,
                       ap=[[0, parts]] + dims)

    with tile.TileContext(nc) as tc:
        with ExitStack() as top:
            const = top.enter_context(tc.tile_pool(name="const", bufs=1))
            ones_r = const.tile([P, 32], f32r, name="ones_r")
            nc.sync.dma_start(out=ones_r, in_=ones_d.ap())
            ident = const.tile([P, P], f32, name="ident")
            make_identity(nc, ident)
            bqk_sb = const.tile([P, 4], f32, name="bqk_sb")
            nc.sync.dma_start(out=bqk_sb, in_=bqk_d.ap())
            bv_sb = const.tile([P, 2], f32, name="bv_sb")
            nc.sync.dma_start(out=bv_sb, in_=bv_d.ap())
            eps_sb = const.tile([P, 1], f32, name="eps_sb")
            nc.vector.memset(eps_sb, EPS)

            dram = top.enter_context(tc.tile_pool(name="dram", bufs=1, space="DRAM"))

            with ExitStack() as phase1:
                # phase-1 activations (freed after the AllToAll send)
                p1 = phase1.enter_context(tc.tile_pool(name="p1", bufs=1))
                qk_sb = [p1.tile([P, N], f32r, name=f"qk{m}") for m in range(4)]
                v_sb = [p1.tile([P, 4, 65], f32r, name=f"v{m}") for m in range(MC16)]
                o_sb = [p1.tile([P, N], f32r, name=f"o{pr}") for pr in range(2)]
                rstd_bc = p1.tile([P, N], f32, name="rstd_bc")
                rstd_col = p1.tile([P, MC16], f32, name="rstd_col")

                # ---------------- LN1 stats + qkv ----------------
                with ExitStack() as ph1:
                    xt_pool = ph1.enter_context(tc.tile_pool(name="xt", bufs=1))
                    w_pool = ph1.enter_context(tc.tile_pool(name="wqkv", bufs=1))
                    st_pool = ph1.enter_context(tc.tile_pool(name="st", bufs=2))

                    xt = []
                    for c in range(C8):
                        t = xt_pool.tile([P, N], f32r, name=f"xt{c}")
                        nc.sync.dma_start(out=t, in_=xT_d.ap()[c * P:(c + 1) * P, :])
                        xt.append(t)
                    wqk = []
                    for c in range(C8):
                        t = w_pool.tile([P, 512], f32r, name=f"wqk{c}")
                        nc.sync.dma_start(out=t, in_=wqk_d.ap()[c * P:(c + 1) * P, :])
                        wqk.append(t)
                    wv = []
                    for c in range(C8):
                        t = w_pool.tile([P, 256], f32r, name=f"wv{c}")
                        nc.sync.dma_start(out=t, in_=wv_d.ap()[c * P:(c + 1) * P, :])
                        wv.append(t)

                    # stats: S1 = sum_c x, S2 = sum_c x^2 (column sums via ones-matmul)
                    with ExitStack() as stc:
                        sps = stc.enter_context(
                            tc.tile_pool(name="sps", bufs=1, space="PSUM"))
                        s1p = [sps.tile([32, 512], f32, name=f"s1p{nt}", tag=f"s1p{nt}")
                               for nt in range(NT)]
                        s2p = [sps.tile([32, 512], f32, name=f"s2p{nt}", tag=f"s2p{nt}")
                               for nt in range(NT)]
                        for c in range(C8):
                            sq_t = st_pool.tile([P, N], f32r, name="sq", tag="sq")
                            nc.scalar.activation(out=sq_t, in_=xt[c].bitcast(f32),
                                                 func=AF.Square)
                            for nt in range(NT):
                                nsl = slice(nt * 512, (nt + 1) * 512)
                                nc.tensor.matmul(s1p[nt], ones_r, xt[c][:, nsl],
                                                 start=(c == 0), stop=(c == C8 - 1),
                                                 skip_group_check=True)
                                nc.tensor.matmul(s2p[nt], ones_r, sq_t[:, nsl],
                                                 start=(c == 0), stop=(c == C8 - 1),
                                                 skip_group_check=True)
                        # engines can only address partition bases 0/32/64/96,
                        # so stage the 4 psum rows on one partition and bounce
                        # through DRAM to get a [4, 512] layout
                        st1_row = p1.tile([1, N], f32, name="st1_row")
                        st2_row = p1.tile([1, N], f32, name="st2_row")
                        for nt in range(NT):
                            nsl = slice(nt * 512, (nt + 1) * 512)
                            nc.vector.tensor_copy(st1_row[:, nsl], s1p[nt][0:1, :])
                            nc.vector.tensor_copy(st2_row[:, nsl], s2p[nt][0:1, :])
                        st_d = dram.tile([2, 4, 512], f32, name="st_d")
                        nc.sync.dma_start(
                            out=st_d[0].rearrange("a b -> (a b)"), in_=st1_row)
                        nc.sync.dma_start(
                            out=st_d[1].rearrange("a b -> (a b)"), in_=st2_row)
                        stats1 = p1.tile([4, 512], f32, name="stats1")
                        stats2 = p1.tile([4, 512], f32, name="stats2")
                        nc.sync.dma_start(out=stats1, in_=st_d[0])
                        nc.sync.dma_start(out=stats2, in_=st_d[1])

                    # rstd = 1/sqrt(S2/C - (S1/C)^2 + eps)   on [4, 512]
                    mu4 = p1.tile([4, 512], f32, name="mu4")
                    var4 = p1.tile([4, 512], f32, name="var4")
                    nc.vector.tensor_scalar_mul(mu4, stats1, 1.0 / DIM)
                    nc.vector.tensor_scalar_mul(var4, stats2, 1.0 / DIM)
                    msq = p1.tile([4, 512], f32, name="msq")
                    nc.vector.tensor_mul(msq, mu4, mu4)
                    nc.vector.tensor_sub(var4, var4, msq)
                    sd4 = p1.tile([4, 512], f32, name="sd4")
                    nc.scalar.activation(out=sd4, in_=var4, func=AF.Sqrt,
                                         bias=eps_sb[0:4, :])
                    rstd4 = p1.tile([4, 512], f32, name="rstd4")
                    nc.vector.reciprocal(rstd4, sd4)
                    # broadcast rstd along tokens-on-free (rstd_bc) and
                    # tokens-on-partitions (rstd_col) via a DRAM bounce
                    # (partition-broadcast APs are only legal on DRAM tensors)
                    rs_d = dram.tile([MC16, P], f32, name="rs_d")
                    nc.sync.dma_start(
                        out=rs_d.rearrange("(a b) c -> a (b c)", b=4), in_=rstd4)
                    rs_flat = rs_d.rearrange("a b -> (a b)")
                    for nt in range(NT):
                        nc.sync.dma_start(
                            out=rstd_bc[:, nt * 512:(nt + 1) * 512],
                            in_=bc_row(rs_flat[nt * 512:(nt + 1) * 512], P))
                    nc.sync.dma_start(
                        out=rstd_col, in_=rs_d.rearrange("c p -> p c"))

                    # q,k feature-major: qk_sb[mi] = (W0_qk^T x)[mi]*rstd + b_qk[mi]
                    with ExitStack() as qkc:
                        qps = qkc.enter_context(
                            tc.tile_pool(name="qps", bufs=3, space="PSUM"))
                        for mi in range(4):
                            for nt in range(NT):
                                nsl = slice(nt * 512, (nt + 1) * 512)
                                ps = qps.tile([P, 512], f32, name="qkp", tag="qkp")
                                for c in range(C8):
                                    nc.tensor.matmul(
                                        ps, wqk[c][:, mi * P:(mi + 1) * P],
                                        xt[c][:, nsl],
                                        start=(c == 0), stop=(c == C8 - 1))
                                nc.vector.tensor_tensor(
                                    out=qk_sb[mi][:, nsl],
                                    in0=ps, in1=rstd_bc[:, nsl], op=ALU.mult)
                            nc.vector.tensor_scalar_add(
                                qk_sb[mi], qk_sb[mi].bitcast(f32),
                                bqk_sb[:, mi:mi + 1])
                        # v token-major: v_sb[mc] = (x^T W0_v)[mc] * rstd_col[mc]
                        for mc in range(MC16):
                            ps = qps.tile([P, 256], f32, name="vp", tag="vp", bufs=2)
                            for c in range(C8):
                                nc.tensor.matmul(ps, xt[c][:, mc * P:(mc + 1) * P],
                                                 wv[c],
                                                 start=(c == 0), stop=(c == C8 - 1))
                            nc.vector.tensor_scalar_mul(
                                v_sb[mc][:, :, 0:64],
                                ps.rearrange("p (h d) -> p h d", h=4),
                                rstd_col[:, mc:mc + 1])
                            nc.vector.memset(v_sb[mc].bitcast(f32)[:, :, 64:65], 1.0)

                # ---------------- attention ----------------
                with ExitStack() as ph2:
                    e_pool = ph2.enter_context(tc.tile_pool(name="epool", bufs=3))
                    r_pool = ph2.enter_context(tc.tile_pool(name="rpool", bufs=2))
                    aps = ph2.enter_context(
                        tc.tile_pool(name="aps", bufs=1, space="PSUM"))
                    for nt in range(NT):
                        nsl = slice(nt * 512, (nt + 1) * 512)
                        o_ps = [aps.tile([65, 512], f32, name=f"ops{h}",
                                         tag=f"ops{h}") for h in range(4)]
                        # software pipeline: AV for chunk mc-1 is issued after
                        # QK/exp for chunk mc, so the PE never waits on ACT
                        prev_e = [None] * 4
                        for mc in range(MC16 + 1):
                            if mc < MC16:
                                msl = slice(mc * P, (mc + 1) * P)
                                cur_e = []
                                for h in range(4):
                                    pr, hh = divmod(h, 2)
                                    qt, kt = qk_sb[pr], qk_sb[2 + pr]
                                    hsl = slice(hh * 64, (hh + 1) * 64)
                                    s_ps = aps.tile([P, 512], f32, name=f"sps{h}",
                                                    tag=f"sps{h}")
                                    nc.tensor.matmul(s_ps, kt[hsl, msl],
                                                     qt[hsl, nsl],
                                                     start=True, stop=True)
                                    e_t = e_pool.tile([P, 512], f32r,
                                                      name=f"e{h}", tag=f"e{h}")
                                    nc.scalar.activation(out=e_t, in_=s_ps,
                                                         func=AF.Exp,
                                                         scale=float(HD) ** -0.5)
                                    cur_e.append(e_t)
                            if mc > 0:
                                # [V_h | 1] x E_h: rows 0:64 = o_h, row 64 = denom
                                for h in range(4):
                                    nc.tensor.matmul(
                                        o_ps[h], v_sb[mc - 1][:, h, :], prev_e[h],
                                        start=(mc == 1), stop=(mc == MC16),
                                        skip_group_check=True)
                            if mc < MC16:
                                prev_e = cur_e
                        rv_d = dram.tile([4, 512], f32, name="rv_d", tag="rv_d",
                                         bufs=2)
                        for h in range(4):
                            rinv = r_pool.tile([1, 512], f32, name="rinv",
                                               tag="rinv")
                            nc.vector.reciprocal(rinv, o_ps[h][64:65, :])
                            nc.sync.dma_start(out=rv_d[h:h + 1, :], in_=rinv)
                        for h in range(4):
                            pr, hh = divmod(h, 2)
                            rbc = r_pool.tile([64, 512], f32, name=f"rbc{h % 2}",
                                              tag=f"rbc{h % 2}")
                            nc.sync.dma_start(
                                out=rbc, in_=bc_row(rv_d[h:h + 1, :], 64))
                            nc.vector.tensor_tensor(
                                out=o_sb[pr][hh * 64:(hh + 1) * 64, nsl],
                                in0=o_ps[h][0:64, :], in1=rbc, op=ALU.mult)
                    for pr in range(2):
                        nc.vector.tensor_scalar_add(
                            o_sb[pr], o_sb[pr].bitcast(f32), bv_sb[:, pr:pr + 1])

                # ---------------- AllToAll send ----------------
                # 8-core AllToAll: block j carries my 4 heads for the 256-token
                # slice of MY batch that core j will own in phase 2.
                send = dram.tile([8, 256, 256], f32r, name="a2a_send")
                for j in range(8):
                    for pr in range(2):
                        nc.sync.dma_start(
                            out=send[j, pr * P:(pr + 1) * P, :],
                            in_=o_sb[pr][:, j * 256:(j + 1) * 256])

            recv = dram.tile([8, 256, 256], f32r, name="a2a_recv")
            nc.gpsimd.collective_compute(
                "AllToAll", mybir.AluOpType.bypass,
                replica_groups=[list(range(8))],
                ins=[send.opt()], outs=[recv.opt()])

            # ---------------- phase 2: proj + residual + LN2 ----------------
            # phase-2 tokens on this core: 256 of batch 0 (cols 0:256) then
            # 256 of batch 1 (cols 256:512); head-chunk c of the gathered
            # o^T comes from group-rank c//2, pair c%2, per batch.
            p2 = top.enter_context(tc.tile_pool(name="p2", bufs=1))
            of_T = [p2.tile([P, 512], f32r, name=f"of{i}") for i in range(C8)]
            for i in range(C8):
                for g in range(2):
                    nc.sync.dma_start(
                        out=of_T[i][:, g * 256:(g + 1) * 256],
                        in_=recv[g * 4 + i // 2, (i % 2) * P:(i % 2 + 1) * P, :])
            xpb = [p2.tile([P, DIM], f32, name=f"xpb{n4}") for n4 in range(4)]
            for n4 in range(4):
                nc.sync.dma_start(out=xpb[n4],
                                  in_=xpb_d.ap()[n4 * P:(n4 + 1) * P, :])
            x2_sb = [p2.tile([P, DIM], f32, name=f"x2{n4}") for n4 in range(4)]
            h2T = [p2.tile([P, TOKS], f32r, name=f"h2T{c}") for c in range(C8)]

            with ExitStack() as ph4:
                wp_pool = ph4.enter_context(tc.tile_pool(name="wp", bufs=4))
                h2_pool = ph4.enter_context(tc.tile_pool(name="h2p", bufs=2))
                pps = ph4.enter_context(tc.tile_pool(name="pps", bufs=1, space="PSUM"))
                tps = ph4.enter_context(tc.tile_pool(name="tps", bufs=2, space="PSUM"))
                for ct in range(2):
                    csl = slice(ct * 512, (ct + 1) * 512)
                    pj_ps = [pps.tile([P, 512], f32, name=f"pj{n4}", tag=f"pj{n4}")
                             for n4 in range(4)]
                    for hdc in range(C8):
                        wpt = wp_pool.tile([P, 512], f32r, name="wpt", tag="wpt")
                        nc.sync.dma_start(
                            out=wpt, in_=wp_d.ap()[hdc * P:(hdc + 1) * P, csl])
                        for n4 in range(4):
                            nc.tensor.matmul(pj_ps[n4],
                                             of_T[hdc][:, n4 * P:(n4 + 1) * P], wpt,
                                             start=(hdc == 0), stop=(hdc == C8 - 1))
                    for n4 in range(4):
                        nc.vector.tensor_tensor(out=x2_sb[n4][:, csl],
                                                in0=pj_ps[n4], in1=xpb[n4][:, csl],
                                                op=ALU.add)
                # LN2 (token-major, bn_stats) + transpose to h2T
                for n4 in range(4):
                    st = h2_pool.tile([P, 2, 6], f32, name="bnst", tag="bnst")
                    for g in range(2):
                        nc.vector.bn_stats(out=st[:, g, :],
                                           in_=x2_sb[n4][:, g * 512:(g + 1) * 512])
                    mv = h2_pool.tile([P, 2], f32, name="bnmv", tag="bnmv")
                    nc.vector.bn_aggr(out=mv, in_=st)
                    sd = h2_pool.tile([P, 1], f32, name="sd2", tag="sd2")
                    nc.scalar.activation(out=sd, in_=mv[:, 1:2], func=AF.Sqrt,
                                         bias=eps_sb)
                    rstd2 = h2_pool.tile([P, 1], f32, name="rstd2", tag="rstd2")
                    nc.vector.reciprocal(rstd2, sd)
                    h2t_ = h2_pool.tile([P, DIM], f32, name="h2t_", tag="h2t_")
                    nc.vector.tensor_scalar(out=h2t_, in0=x2_sb[n4],
                                            scalar1=mv[:, 0:1], scalar2=rstd2,
                                            op0=ALU.subtract, op1=ALU.mult)
                    for c in range(C8):
                        tp = tps.tile([P, P], f32, name="tp", tag="tp")
                        nc.tensor.transpose(tp, h2t_[:, c * P:(c + 1) * P], ident)
                        nc.vector.tensor_copy(h2T[c][:, n4 * P:(n4 + 1) * P], tp)

            # ---------------- MLP ----------------
            bf1_sb = p2.tile([P, FF32], f32, name="bf1_sb")
            nc.sync.dma_start(out=bf1_sb, in_=bf1_d.ap())
            g_T = [p2.tile([P, TOKS], f32r, name=f"gT{f}") for f in range(FF32)]
            yout = [p2.tile([P, DIM], f32, name=f"yo{n4}") for n4 in range(4)]
            with ExitStack() as ph5:
                w1_pool = ph5.enter_context(tc.tile_pool(name="w1", bufs=6))
                f1ps = ph5.enter_context(tc.tile_pool(name="f1ps", bufs=1,
                                                      space="PSUM"))
                for ffg in range(8):
                    f1_ps = [f1ps.tile([P, 512], f32, name=f"f1p{f}", tag=f"f1p{f}")
                             for f in range(4)]
                    for c in range(C8):
                        w1t = w1_pool.tile([P, 512], f32r, name="w1t", tag="w1t")
                        nc.sync.dma_start(
                            out=w1t, in_=wf1_d.ap()[c * P:(c + 1) * P,
                                                    ffg * 512:(ffg + 1) * 512])
                        for f in range(4):
                            nc.tensor.matmul(f1_ps[f], w1t[:, f * P:(f + 1) * P],
                                             h2T[c],
                                             start=(c == 0), stop=(c == C8 - 1))
                    for f in range(4):
                        ffc = ffg * 4 + f
                        nc.scalar.activation(out=g_T[ffc], in_=f1_ps[f],
                                             func=AF.Gelu,
                                             bias=bf1_sb[:, ffc:ffc + 1])
            with ExitStack() as ph6:
                w2_pool = ph6.enter_context(tc.tile_pool(name="w2", bufs=6))
                f2ps = ph6.enter_context(tc.tile_pool(name="f2ps", bufs=1,
                                                      space="PSUM"))
                for ct in range(2):
                    csl = slice(ct * 512, (ct + 1) * 512)
                    f2_ps = [f2ps.tile([P, 512], f32, name=f"f2p{n4}", tag=f"f2p{n4}")
                             for n4 in range(4)]
                    for ffc in range(FF32):
                        w2t = w2_pool.tile([P, 512], f32r, name="w2t", tag="w2t")
                        nc.sync.dma_start(
                            out=w2t, in_=wf2_d.ap()[ffc * P:(ffc + 1) * P, csl])
                        for n4 in range(4):
                            nc.tensor.matmul(f2_ps[n4],
                                             g_T[ffc][:, n4 * P:(n4 + 1) * P], w2t,
                                             start=(ffc == 0), stop=(ffc == FF32 - 1))
                    for n4 in range(4):
                        nc.vector.tensor_tensor(out=yout[n4][:, csl],
                                                in0=f2_ps[n4], in1=x2_sb[n4][:, csl],
                                                op=ALU.add)
            for n4 in range(4):
                nc.sync.dma_start(out=yout_d.ap()[n4 * P:(n4 + 1) * P, :],
                                  in_=yout[n4])

    nc.compile()
    _CACHE["nc"] = nc
    return nc


def _prep_inputs(x, ln1_g, ln1_b, qkv_w, proj_w, proj_b, ln2_g, ln2_b,
                 fc1_w, fc1_b, fc2_w, fc2_b):
    """Host-side sharding + weight folding. Returns list of 8 in_maps."""
    f4 = np.float32
    x = np.asarray(x, f4)
    qkv_w = np.asarray(qkv_w, f4)
    w1 = qkv_w * np.asarray(ln1_g, f4)[:, None]          # gamma fold
    w1c = w1 - w1.mean(axis=0, keepdims=True)            # mean-centering fold
    bqkv = np.asarray(ln1_b, f4) @ qkv_w                 # beta fold -> bias [3*DIM]
    w1c_r = w1c.reshape(DIM, 3, HEADS, HD)
    bqkv_r = bqkv.reshape(3, HEADS, HD)

    wf1 = np.asarray(fc1_w, f4) * np.asarray(ln2_g, f4)[:, None]
    wf1c = np.ascontiguousarray(wf1 - wf1.mean(axis=0, keepdims=True))
    bf1 = np.asarray(fc1_b, f4) + np.asarray(ln2_b, f4) @ np.asarray(fc1_w, f4)

    in_maps = []
    for core in range(NCORES):
        g, j = divmod(core, 4)
        h0 = 4 * j
        wq = w1c_r[:, 0, h0:h0 + 4, :].reshape(DIM, 256)
        wk = w1c_r[:, 1, h0:h0 + 4, :].reshape(DIM, 256)
        wv = w1c_r[:, 2, h0:h0 + 4, :].reshape(DIM, 256)
        bq = bqkv_r[0, h0:h0 + 4, :].reshape(256)
        bk = bqkv_r[1, h0:h0 + 4, :].reshape(256)
        bv = bqkv_r[2, h0:h0 + 4, :].reshape(256)
        in_maps.append({
            "xT": np.ascontiguousarray(x[g].T),
            "w_qk": np.ascontiguousarray(np.concatenate([wq, wk], axis=1)),
            "w_v": np.ascontiguousarray(wv),
            "b_qk": np.ascontiguousarray(
                np.concatenate([bq, bk]).reshape(4, P).T),
            "b_v": np.ascontiguousarray(bv.reshape(2, P).T),
            "w_p": np.asarray(proj_w, f4),
            "x_pb": np.ascontiguousarray(
                np.concatenate([x[0, core * 256:(core + 1) * 256, :],
                                x[1, core * 256:(core + 1) * 256, :]], axis=0)
                + np.asarray(proj_b, f4)),
            "w_f1": wf1c,
            "b_f1": np.ascontiguousarray(bf1.reshape(FF32, P).T),
            "w_f2": np.asarray(fc2_w, f4),
            "ones_c": np.ones((P, 32), f4),
        })
    return in_maps


def kernel(**inputs):
    from concourse.bass_utils import run_bass_kernel_spmd

    nc = _build()
    in_maps = _prep_inputs(**inputs)
    res = run_bass_kernel_spmd(nc, in_maps, core_ids=list(range(NCORES)))
    return assemble_output(res.results, inputs)


def assemble_output(results, inputs):
    fc2_b = np.asarray(inputs["fc2_b"], np.float32)
    out = np.empty((B, N, DIM), np.float32)
    for core in range(NCORES):
        y = results[core]["y_out"] + fc2_b
        out[0, core * 256:(core + 1) * 256, :] = y[0:256]
        out[1, core * 256:(core + 1) * 256, :] = y[256:512]
    return out


# revision 23
# speedup vs baseline: 1.2698x; 1.2620x over previous
"""Trainium2 Bass kernel for a pre-norm transformer block (B=2, N=2048, D=1024, H=16, FF=4096).

Strategy (8 cores):
  Phase 1 (DP on batch x TP on heads): cores 0-3 handle batch 0, cores 4-7 batch 1;
    each core computes LN1 + qkv + attention for its 4 heads over all 2048 tokens,
    in feature-major ("transposed") layout so no on-chip transposes are needed.
    LN1 is folded into the weights host-side (gamma-scaling + mean-centering of the
    qkv weight columns); the per-token rstd is applied after the matmul via a
    DMA-broadcast row. Softmax runs without max-subtraction (scores are O(10), fp32
    exp is safe); denominators come from column-packed ones-matmuls.
  Reshard: one AllToAll per 4-core group moves head-shards -> token-shards.
  Phase 2 (token-parallel): each core runs proj + residual + LN2 + MLP for its 512
    tokens with full weights. proj_b is pre-added to the residual host-side; ln2_g/b
    are folded into fc1_w/fc1_b host-side; fc2_b is added host-side after gather.

All matmuls run in float32r (full PE rate at N>=256, ~1e-3 matmul accuracy).
"""

import numpy as np

DIM = 1024
HEADS = 16
HD = 64
FF = 4096
B = 2
N = 2048
EPS = 1e-5
P = 128
NCORES = 8
GROUPS = [[0, 1, 2, 3], [4, 5, 6, 7]]
TOKS = 512         # tokens per core in phase 2
C8 = DIM // P      # 8 contraction chunks
NT = N // 512      # 4 n-tiles
MC16 = N // P      # 16 m-chunks
FF32 = FF // P     # 32 ff chunks

_CACHE = {}


def _build():
    if "nc" in _CACHE:
        return _CACHE["nc"]

    import concourse.bacc as bacc
    import concourse.bass as bass
    import concourse.tile as tile
    from concourse import mybir
    from concourse.masks import make_identity
    from contextlib import ExitStack

    f32 = mybir.dt.float32
    f32r = mybir.dt.float32r
    AF = mybir.ActivationFunctionType
    ALU = mybir.AluOpType

    nc = bacc.Bacc("TRN2", target_bir_lowering=False, debug=False,
                   num_devices=NCORES)

    # ---- per-core dram tensors ----
    xT_d = nc.dram_tensor("xT", [DIM, N], f32r, kind="ExternalInput")
    wqk_d = nc.dram_tensor("w_qk", [DIM, 512], f32r, kind="ExternalInput")
    wv_d = nc.dram_tensor("w_v", [DIM, 256], f32r, kind="ExternalInput")
    bqk_d = nc.dram_tensor("b_qk", [P, 4], f32, kind="ExternalInput")
    bv_d = nc.dram_tensor("b_v", [P, 2], f32, kind="ExternalInput")
    wp_d = nc.dram_tensor("w_p", [DIM, DIM], f32r, kind="ExternalInput")
    xpb_d = nc.dram_tensor("x_pb", [TOKS, DIM], f32, kind="ExternalInput")
    wf1_d = nc.dram_tensor("w_f1", [DIM, FF], f32r, kind="ExternalInput")
    bf1_d = nc.dram_tensor("b_f1", [P, FF32], f32, kind="ExternalInput")
    wf2_d = nc.dram_tensor("w_f2", [FF, DIM], f32r, kind="ExternalInput")
    ones_d = nc.dram_tensor("ones_c", [P, 32], f32r, kind="ExternalInput")
    yout_d = nc.dram_tensor("y_out", [TOKS, DIM], f32, kind="ExternalOutput")

    def bc_row(ap_row, parts):
        # partition-broadcast AP for DMA: read one row into `parts` partitions
        t = ap_row
        dims = [list(d) for d in t.ap]
        if dims and dims[0][1] == 1:
            dims = dims[1:]
        return bass.AP(tensor=t.tensor, offset=t.offset,
                       ap=[[0, parts]] + dims)

    with tile.TileContext(nc) as tc:
        with ExitStack() as top:
            const = top.enter_context(tc.tile_pool(name="const", bufs=1))
            ones_r = const.tile([P, 32], f32r, name="ones_r")
            nc.sync.dma_start(out=ones_r, in_=ones_d.ap())
            ident = const.tile([P, P], f32, name="ident")
            make_identity(nc, ident)
            bqk_sb = const.tile([P, 4], f32, name="bqk_sb")
            nc.sync.dma_start(out=bqk_sb, in_=bqk_d.ap())
            bv_sb = const.tile([P, 2], f32, name="bv_sb")
            nc.sync.dma_start(out=bv_sb, in_=bv_d.ap())
            eps_sb = const.tile([P, 1], f32, name="eps_sb")
            nc.vector.memset(eps_sb, EPS)

            dram = top.enter_context(tc.tile_pool(name="dram", bufs=1, space="DRAM"))

            with ExitStack() as phase1:
                # phase-1 activations (freed after the AllToAll send)
                p1 = phase1.enter_context(tc.tile_pool(name="p1", bufs=1))
                # q pair tiles (2 heads stacked) + per-head zero-padded K^T:
                # khat[h] has k_h in rows (h%2)*64..+64 and zeros elsewhere, so
                # QK can contract over the full 128 partitions (K=64 matmuls
                # hold the PE at half clock - HAM sees a half-idle array).
                q_sb = [p1.tile([P, N], f32r, name=f"q{m}") for m in range(2)]
                khat = [p1.tile([P, N], f32r, name=f"kh{h}") for h in range(4)]
                v_sb = [p1.tile([P, 4, 65], f32r, name=f"v{m}") for m in range(MC16)]
                rstd_bc = p1.tile([P, N], f32, name="rstd_bc")
                rstd_col = p1.tile([P, MC16], f32, name="rstd_col")

                # ---------------- LN1 stats + qkv ----------------
                with ExitStack() as ph1:
                    xt_pool = ph1.enter_context(tc.tile_pool(name="xt", bufs=1))
                    w_pool = ph1.enter_context(tc.tile_pool(name="wqkv", bufs=1))
                    st_pool = ph1.enter_context(tc.tile_pool(name="st", bufs=1))

                    xt = []
                    for c in range(C8):
                        t = xt_pool.tile([P, N], f32r, name=f"xt{c}")
                        nc.sync.dma_start(out=t, in_=xT_d.ap()[c * P:(c + 1) * P, :])
                        xt.append(t)
                    wqk = []
                    for c in range(C8):
                        t = w_pool.tile([P, 512], f32r, name=f"wqk{c}")
                        nc.sync.dma_start(out=t, in_=wqk_d.ap()[c * P:(c + 1) * P, :])
                        wqk.append(t)
                    wv = []
                    for c in range(C8):
                        t = w_pool.tile([P, 256], f32r, name=f"wv{c}")
                        nc.sync.dma_start(out=t, in_=wv_d.ap()[c * P:(c + 1) * P, :])
                        wv.append(t)

                    # stats: S1 = sum_c x, S2 = sum_c x^2 (column sums via ones-matmul)
                    with ExitStack() as stc:
                        sps = stc.enter_context(
                            tc.tile_pool(name="sps", bufs=1, space="PSUM"))
                        s1p = [sps.tile([32, 512], f32, name=f"s1p{nt}", tag=f"s1p{nt}")
                               for nt in range(NT)]
                        s2p = [sps.tile([32, 512], f32, name=f"s2p{nt}", tag=f"s2p{nt}")
                               for nt in range(NT)]
                        for c in range(C8):
                            sq_t = st_pool.tile([P, N], f32r, name="sq", tag="sq")
                            nc.scalar.activation(out=sq_t, in_=xt[c].bitcast(f32),
                                                 func=AF.Square)
                            for nt in range(NT):
                                nsl = slice(nt * 512, (nt + 1) * 512)
                                nc.tensor.matmul(s1p[nt], ones_r, xt[c][:, nsl],
                                                 start=(c == 0), stop=(c == C8 - 1),
                                                 skip_group_check=True)
                                nc.tensor.matmul(s2p[nt], ones_r, sq_t[:, nsl],
                                                 start=(c == 0), stop=(c == C8 - 1),
                                                 skip_group_check=True)
                        # engines can only address partition bases 0/32/64/96,
                        # so stage each psum row through a small sbuf tile and
                        # bounce via DRAM into a [4, 512] layout
                        st_d = dram.tile([2, 4, 512], f32, name="st_d")
                        for nt in range(NT):
                            for k, sp in ((0, s1p[nt]), (1, s2p[nt])):
                                stg = st_pool.tile([1, 512], f32, name="stg",
                                                   tag="stg", bufs=4)
                                nc.vector.tensor_copy(stg, sp[0:1, :])
                                nc.sync.dma_start(out=st_d[k, nt], in_=stg)
                        stats1 = p1.tile([4, 512], f32, name="stats1")
                        stats2 = p1.tile([4, 512], f32, name="stats2")
                        nc.sync.dma_start(out=stats1, in_=st_d[0])
                        nc.sync.dma_start(out=stats2, in_=st_d[1])

                    # rstd = 1/sqrt(S2/C - (S1/C)^2 + eps)   on [4, 512]
                    mu4 = p1.tile([4, 512], f32, name="mu4")
                    var4 = p1.tile([4, 512], f32, name="var4")
                    nc.vector.tensor_scalar_mul(mu4, stats1, 1.0 / DIM)
                    nc.vector.tensor_scalar_mul(var4, stats2, 1.0 / DIM)
                    msq = p1.tile([4, 512], f32, name="msq")
                    nc.vector.tensor_mul(msq, mu4, mu4)
                    nc.vector.tensor_sub(var4, var4, msq)
                    sd4 = p1.tile([4, 512], f32, name="sd4")
                    nc.scalar.activation(out=sd4, in_=var4, func=AF.Sqrt,
                                         bias=eps_sb[0:4, :])
                    rstd4 = p1.tile([4, 512], f32, name="rstd4")
                    nc.vector.reciprocal(rstd4, sd4)
                    # broadcast rstd along tokens-on-free (rstd_bc) and
                    # tokens-on-partitions (rstd_col) via a DRAM bounce
                    # (partition-broadcast APs are only legal on DRAM tensors)
                    rs_d = dram.tile([MC16, P], f32, name="rs_d")
                    nc.sync.dma_start(
                        out=rs_d.rearrange("(a b) c -> a (b c)", b=4), in_=rstd4)
                    rs_flat = rs_d.rearrange("a b -> (a b)")
                    for nt in range(NT):
                        nc.sync.dma_start(
                            out=rstd_bc[:, nt * 512:(nt + 1) * 512],
                            in_=bc_row(rs_flat[nt * 512:(nt + 1) * 512], P))
                    nc.sync.dma_start(
                        out=rstd_col, in_=rs_d.rearrange("c p -> p c"))

                    # q,k feature-major: qk_sb[mi] = (W0_qk^T x)[mi]*rstd + b_qk[mi]
                    with ExitStack() as qkc:
                        qps = qkc.enter_context(
                            tc.tile_pool(name="qps", bufs=3, space="PSUM"))
                        for h in range(4):
                            nc.vector.memset(
                                khat[h].bitcast(f32)[(1 - h % 2) * 64:
                                                     (2 - h % 2) * 64, :], 0.0)
                        for mi in range(4):
                            for nt in range(NT):
                                nsl = slice(nt * 512, (nt + 1) * 512)
                                ps = qps.tile([P, 512], f32, name="qkp", tag="qkp")
                                for c in range(C8):
                                    nc.tensor.matmul(
                                        ps, wqk[c][:, mi * P:(mi + 1) * P],
                                        xt[c][:, nsl],
                                        start=(c == 0), stop=(c == C8 - 1))
                                if mi < 2:
                                    nc.vector.tensor_tensor(
                                        out=q_sb[mi][:, nsl],
                                        in0=ps, in1=rstd_bc[:, nsl], op=ALU.mult)
                                else:
                                    for hh in range(2):
                                        h = (mi - 2) * 2 + hh
                                        psl = slice(hh * 64, (hh + 1) * 64)
                                        nc.vector.tensor_tensor(
                                            out=khat[h][psl, nsl],
                                            in0=ps[psl, :],
                                            in1=rstd_bc[psl, nsl], op=ALU.mult)
                        for mi in range(2):
                            nc.vector.tensor_scalar_add(
                                q_sb[mi], q_sb[mi].bitcast(f32),
                                bqk_sb[:, mi:mi + 1])
                        for h in range(4):
                            psl = slice((h % 2) * 64, (h % 2 + 1) * 64)
                            nc.vector.tensor_scalar_add(
                                khat[h][psl, :], khat[h].bitcast(f32)[psl, :],
                                bqk_sb[psl, 2 + h // 2:3 + h // 2])
                        # v token-major: v_sb[mc] = (x^T W0_v)[mc] * rstd_col[mc]
                        for mc in range(MC16):
                            ps = qps.tile([P, 256], f32, name="vp", tag="vp", bufs=2)
                            for c in range(C8):
                                nc.tensor.matmul(ps, xt[c][:, mc * P:(mc + 1) * P],
                                                 wv[c],
                                                 start=(c == 0), stop=(c == C8 - 1))
                            nc.vector.tensor_scalar_mul(
                                v_sb[mc][:, :, 0:64],
                                ps.rearrange("p (h d) -> p h d", h=4),
                                rstd_col[:, mc:mc + 1])
                            nc.vector.memset(v_sb[mc].bitcast(f32)[:, :, 64:65], 1.0)

                # ---------------- attention ----------------
                # allocated after the xt pool closes so the slots reuse its space
                o_sb = [p1.tile([P, N], f32r, name=f"o{pr}") for pr in range(2)]
                with ExitStack() as ph2:
                    e_pool = ph2.enter_context(tc.tile_pool(name="epool", bufs=3))
                    r_pool = ph2.enter_context(tc.tile_pool(name="rpool", bufs=2))
                    aps = ph2.enter_context(
                        tc.tile_pool(name="aps", bufs=1, space="PSUM"))
                    for nt in range(NT):
                        nsl = slice(nt * 512, (nt + 1) * 512)
                        o_ps = [aps.tile([65, 512], f32, name=f"ops{h}",
                                         tag=f"ops{h}") for h in range(4)]
                        # software pipeline: AV for chunk mc-1 is issued after
                        # QK/exp for chunk mc, so the PE never waits on ACT
                        prev_e = [None] * 4
                        for mc in range(MC16 + 1):
                            if mc < MC16:
                                msl = slice(mc * P, (mc + 1) * P)
                                cur_e = []
                                for h in range(4):
                                    pr, hh = divmod(h, 2)
                                    s_ps = aps.tile([P, 512], f32, name=f"sps{h}",
                                                    tag=f"sps{h}")
                                    nc.tensor.matmul(s_ps, khat[h][:, msl],
                                                     q_sb[pr][:, nsl],
                                                     start=True, stop=True)
                                    e_t = e_pool.tile([P, 512], f32r,
                                                      name=f"e{h}", tag=f"e{h}")
                                    nc.scalar.activation(out=e_t, in_=s_ps,
                                                         func=AF.Exp,
                                                         scale=float(HD) ** -0.5)
                                    cur_e.append(e_t)
                            if mc > 0:
                                # [V_h | 1] x E_h: rows 0:64 = o_h, row 64 = denom
                                for h in range(4):
                                    nc.tensor.matmul(
                                        o_ps[h], v_sb[mc - 1][:, h, :], prev_e[h],
                                        start=(mc == 1), stop=(mc == MC16),
                                        skip_group_check=True)
                            if mc < MC16:
                                prev_e = cur_e
                        rv_d = dram.tile([4, 512], f32, name="rv_d", tag="rv_d",
                                         bufs=2)
                        for h in range(4):
                            rinv = r_pool.tile([1, 512], f32, name="rinv",
                                               tag="rinv")
                            nc.vector.reciprocal(rinv, o_ps[h][64:65, :])
                            nc.sync.dma_start(out=rv_d[h:h + 1, :], in_=rinv)
                        for h in range(4):
                            pr, hh = divmod(h, 2)
                            rbc = r_pool.tile([64, 512], f32, name=f"rbc{h % 2}",
                                              tag=f"rbc{h % 2}")
                            nc.sync.dma_start(
                                out=rbc, in_=bc_row(rv_d[h:h + 1, :], 64))
                            nc.vector.tensor_tensor(
                                out=o_sb[pr][hh * 64:(hh + 1) * 64, nsl],
                                in0=o_ps[h][0:64, :], in1=rbc, op=ALU.mult)
                    for pr in range(2):
                        nc.vector.tensor_scalar_add(
                            o_sb[pr], o_sb[pr].bitcast(f32), bv_sb[:, pr:pr + 1])

                # ---------------- AllToAll send ----------------
                # 8-core AllToAll: block j carries my 4 heads for the 256-token
                # slice of MY batch that core j will own in phase 2.
                send = dram.tile([8, 256, 256], f32r, name="a2a_send")
                for j in range(8):
                    for pr in range(2):
                        nc.sync.dma_start(
                            out=send[j, pr * P:(pr + 1) * P, :],
                            in_=o_sb[pr][:, j * 256:(j + 1) * 256])

            recv = dram.tile([8, 256, 256], f32r, name="a2a_recv")
            nc.gpsimd.collective_compute(
                "AllToAll", mybir.AluOpType.bypass,
                replica_groups=[list(range(8))],
                ins=[send.opt()], outs=[recv.opt()])

            # ---------------- phase 2: proj + residual + LN2 ----------------
            # phase-2 tokens on this core: 256 of batch 0 (cols 0:256) then
            # 256 of batch 1 (cols 256:512); head-chunk c of the gathered
            # o^T comes from group-rank c//2, pair c%2, per batch.
            p2 = top.enter_context(tc.tile_pool(name="p2", bufs=1))
            of_T = [p2.tile([P, 512], f32r, name=f"of{i}") for i in range(C8)]
            for i in range(C8):
                for g in range(2):
                    nc.sync.dma_start(
                        out=of_T[i][:, g * 256:(g + 1) * 256],
                        in_=recv[g * 4 + i // 2, (i % 2) * P:(i % 2 + 1) * P, :])
            xpb = [p2.tile([P, DIM], f32, name=f"xpb{n4}") for n4 in range(4)]
            for n4 in range(4):
                nc.sync.dma_start(out=xpb[n4],
                                  in_=xpb_d.ap()[n4 * P:(n4 + 1) * P, :])
            x2_sb = [p2.tile([P, DIM], f32, name=f"x2{n4}") for n4 in range(4)]
            h2T = [p2.tile([P, TOKS], f32r, name=f"h2T{c}") for c in range(C8)]

            with ExitStack() as ph4:
                wp_pool = ph4.enter_context(tc.tile_pool(name="wp", bufs=4))
                h2_pool = ph4.enter_context(tc.tile_pool(name="h2p", bufs=2))
                pps = ph4.enter_context(tc.tile_pool(name="pps", bufs=1, space="PSUM"))
                tps = ph4.enter_context(tc.tile_pool(name="tps", bufs=2, space="PSUM"))
                for ct in range(2):
                    csl = slice(ct * 512, (ct + 1) * 512)
                    pj_ps = [pps.tile([P, 512], f32, name=f"pj{n4}", tag=f"pj{n4}")
                             for n4 in range(4)]
                    for hdc in range(C8):
                        wpt = wp_pool.tile([P, 512], f32r, name="wpt", tag="wpt")
                        nc.sync.dma_start(
                            out=wpt, in_=wp_d.ap()[hdc * P:(hdc + 1) * P, csl])
                        for n4 in range(4):
                            nc.tensor.matmul(pj_ps[n4],
                                             of_T[hdc][:, n4 * P:(n4 + 1) * P], wpt,
                                             start=(hdc == 0), stop=(hdc == C8 - 1))
                    for n4 in range(4):
                        nc.vector.tensor_tensor(out=x2_sb[n4][:, csl],
                                                in0=pj_ps[n4], in1=xpb[n4][:, csl],
                                                op=ALU.add)
                # LN2 (token-major, bn_stats) + transpose to h2T
                for n4 in range(4):
                    st = h2_pool.tile([P, 2, 6], f32, name="bnst", tag="bnst")
                    for g in range(2):
                        nc.vector.bn_stats(out=st[:, g, :],
                                           in_=x2_sb[n4][:, g * 512:(g + 1) * 512])
                    mv = h2_pool.tile([P, 2], f32, name="bnmv", tag="bnmv")
                    nc.vector.bn_aggr(out=mv, in_=st)
                    sd = h2_pool.tile([P, 1], f32, name="sd2", tag="sd2")
                    nc.scalar.activation(out=sd, in_=mv[:, 1:2], func=AF.Sqrt,
                                         bias=eps_sb)
                    rstd2 = h2_pool.tile([P, 1], f32, name="rstd2", tag="rstd2")
                    nc.vector.reciprocal(rstd2, sd)
                    h2t_ = h2_pool.tile([P, DIM], f32, name="h2t_", tag="h2t_")
                    nc.vector.tensor_scalar(out=h2t_, in0=x2_sb[n4],
                                            scalar1=mv[:, 0:1], scalar2=rstd2,
                                            op0=ALU.subtract, op1=ALU.mult)
                    for c in range(C8):
                        tp = tps.tile([P, P], f32, name="tp", tag="tp")
                        nc.tensor.transpose(tp, h2t_[:, c * P:(c + 1) * P], ident)
                        nc.vector.tensor_copy(h2T[c][:, n4 * P:(n4 + 1) * P], tp)

            # ---------------- MLP ----------------
            bf1_sb = p2.tile([P, FF32], f32, name="bf1_sb")
            nc.sync.dma_start(out=bf1_sb, in_=bf1_d.ap())
            g_T = [p2.tile([P, TOKS], f32r, name=f"gT{f}") for f in range(FF32)]
            yout = [p2.tile([P, DIM], f32, name=f"yo{n4}") for n4 in range(4)]
            with ExitStack() as ph5:
                w1_pool = ph5.enter_context(tc.tile_pool(name="w1", bufs=6))
                f1ps = ph5.enter_context(tc.tile_pool(name="f1ps", bufs=1,
                                                      space="PSUM"))
                for ffg in range(8):
                    f1_ps = [f1ps.tile([P, 512], f32, name=f"f1p{f}", tag=f"f1p{f}")
                             for f in range(4)]
                    for c in range(C8):
                        w1t = w1_pool.tile([P, 512], f32r, name="w1t", tag="w1t")
                        nc.sync.dma_start(
                            out=w1t, in_=wf1_d.ap()[c * P:(c + 1) * P,
                                                    ffg * 512:(ffg + 1) * 512])
                        for f in range(4):
                            nc.tensor.matmul(f1_ps[f], w1t[:, f * P:(f + 1) * P],
                                             h2T[c],
                                             start=(c == 0), stop=(c == C8 - 1))
                    for f in range(4):
                        ffc = ffg * 4 + f
                        nc.scalar.activation(out=g_T[ffc], in_=f1_ps[f],
                                             func=AF.Gelu,
                                             bias=bf1_sb[:, ffc:ffc + 1])
            with ExitStack() as ph6:
                w2_pool = ph6.enter_context(tc.tile_pool(name="w2", bufs=6))
                f2ps = ph6.enter_context(tc.tile_pool(name="f2ps", bufs=1,
                                                      space="PSUM"))
                for ct in range(2):
                    csl = slice(ct * 512, (ct + 1) * 512)
                    f2_ps = [f2ps.tile([P, 512], f32, name=f"f2p{n4}", tag=f"f2p{n4}")
                             for n4 in range(4)]
                    for ffc in range(FF32):
                        w2t = w2_pool.tile([P, 512], f32r, name="w2t", tag="w2t")
                        nc.sync.dma_start(
                            out=w2t, in_=wf2_d.ap()[ffc * P:(ffc + 1) * P, csl])
                        for n4 in range(4):
                            nc.tensor.matmul(f2_ps[n4],
                                             g_T[ffc][:, n4 * P:(n4 + 1) * P], w2t,
                                             start=(ffc == 0), stop=(ffc == FF32 - 1))
                    for n4 in range(4):
                        nc.vector.tensor_tensor(out=yout[n4][:, csl],
                                                in0=f2_ps[n4], in1=x2_sb[n4][:, csl],
                                                op=ALU.add)
            for n4 in range(4):
                nc.sync.dma_start(out=yout_d.ap()[n4 * P:(n4 + 1) * P, :],
                                  in_=yout[n4])

    nc.compile()
    _CACHE["nc"] = nc
    return nc


def _prep_inputs(x, ln1_g, ln1_b, qkv_w, proj_w, proj_b, ln2_g, ln2_b,
                 fc1_w, fc1_b, fc2_w, fc2_b):
    """Host-side sharding + weight folding. Returns list of 8 in_maps."""
    f4 = np.float32
    x = np.asarray(x, f4)
    qkv_w = np.asarray(qkv_w, f4)
    w1 = qkv_w * np.asarray(ln1_g, f4)[:, None]          # gamma fold
    w1c = w1 - w1.mean(axis=0, keepdims=True)            # mean-centering fold
    bqkv = np.asarray(ln1_b, f4) @ qkv_w                 # beta fold -> bias [3*DIM]
    w1c_r = w1c.reshape(DIM, 3, HEADS, HD)
    bqkv_r = bqkv.reshape(3, HEADS, HD)

    wf1 = np.asarray(fc1_w, f4) * np.asarray(ln2_g, f4)[:, None]
    wf1c = np.ascontiguousarray(wf1 - wf1.mean(axis=0, keepdims=True))
    bf1 = np.asarray(fc1_b, f4) + np.asarray(ln2_b, f4) @ np.asarray(fc1_w, f4)

    in_maps = []
    for core in range(NCORES):
        g, j = divmod(core, 4)
        h0 = 4 * j
        wq = w1c_r[:, 0, h0:h0 + 4, :].reshape(DIM, 256)
        wk = w1c_r[:, 1, h0:h0 + 4, :].reshape(DIM, 256)
        wv = w1c_r[:, 2, h0:h0 + 4, :].reshape(DIM, 256)
        bq = bqkv_r[0, h0:h0 + 4, :].reshape(256)
        bk = bqkv_r[1, h0:h0 + 4, :].reshape(256)
        bv = bqkv_r[2, h0:h0 + 4, :].reshape(256)
        in_maps.append({
            "xT": np.ascontiguousarray(x[g].T),
            "w_qk": np.ascontiguousarray(np.concatenate([wq, wk], axis=1)),
            "w_v": np.ascontiguousarray(wv),
            "b_qk": np.ascontiguousarray(
                np.concatenate([bq, bk]).reshape(4, P).T),
            "b_v": np.ascontiguousarray(bv.reshape(2, P).T),
            "w_p": np.asarray(proj_w, f4),
            "x_pb": np.ascontiguousarray(
                np.concatenate([x[0, core * 256:(core + 1) * 256, :],
                                x[1, core * 256:(core + 1) * 256, :]], axis=0)
                + np.asarray(proj_b, f4)),
            "w_f1": wf1c,
            "b_f1": np.ascontiguousarray(bf1.reshape(FF32, P).T),
            "w_f2": np.asarray(fc2_w, f4),
            "ones_c": np.ones((P, 32), f4),
        })
    return in_maps


def kernel(**inputs):
    from concourse.bass_utils import run_bass_kernel_spmd

    nc = _build()
    in_maps = _prep_inputs(**inputs)
    res = run_bass_kernel_spmd(nc, in_maps, core_ids=list(range(NCORES)))
    return assemble_output(res.results, inputs)


def assemble_output(results, inputs):
    fc2_b = np.asarray(inputs["fc2_b"], np.float32)
    out = np.empty((B, N, DIM), np.float32)
    for core in range(NCORES):
        y = results[core]["y_out"] + fc2_b
        out[0, core * 256:(core + 1) * 256, :] = y[0:256]
        out[1, core * 256:(core + 1) * 256, :] = y[256:512]
    return out


# revision 24
# speedup vs baseline: 1.3625x; 1.0730x over previous
"""Trainium2 Bass kernel for a pre-norm transformer block (B=2, N=2048, D=1024, H=16, FF=4096).

Strategy (8 cores):
  Phase 1 (DP on batch x TP on heads): cores 0-3 handle batch 0, cores 4-7 batch 1;
    each core computes LN1 + qkv + attention for its 4 heads over all 2048 tokens,
    in feature-major ("transposed") layout so no on-chip transposes are needed.
    LN1 is folded into the weights host-side (gamma-scaling + mean-centering of the
    qkv weight columns); the per-token rstd is applied after the matmul via a
    DMA-broadcast row. Softmax runs without max-subtraction (scores are O(10), fp32
    exp is safe); denominators come from column-packed ones-matmuls.
  Reshard: one AllToAll per 4-core group moves head-shards -> token-shards.
  Phase 2 (token-parallel): each core runs proj + residual + LN2 + MLP for its 512
    tokens with full weights. proj_b is pre-added to the residual host-side; ln2_g/b
    are folded into fc1_w/fc1_b host-side; fc2_b is added host-side after gather.

All matmuls run in float32r (full PE rate at N>=256, ~1e-3 matmul accuracy).
"""

import numpy as np

DIM = 1024
HEADS = 16
HD = 64
FF = 4096
B = 2
N = 2048
EPS = 1e-5
P = 128
NCORES = 8
GROUPS = [[0, 1, 2, 3], [4, 5, 6, 7]]
TOKS = 512         # tokens per core in phase 2
C8 = DIM // P      # 8 contraction chunks
NT = N // 512      # 4 n-tiles
MC16 = N // P      # 16 m-chunks
FF32 = FF // P     # 32 ff chunks

_CACHE = {}


def _build():
    if "nc" in _CACHE:
        return _CACHE["nc"]

    import concourse.bacc as bacc
    import concourse.bass as bass
    import concourse.tile as tile
    from concourse import mybir
    from concourse.masks import make_identity
    from contextlib import ExitStack

    f32 = mybir.dt.float32
    f32r = mybir.dt.float32r
    AF = mybir.ActivationFunctionType
    ALU = mybir.AluOpType

    nc = bacc.Bacc("TRN2", target_bir_lowering=False, debug=False,
                   num_devices=NCORES)

    # ---- per-core dram tensors ----
    xT_d = nc.dram_tensor("xT", [DIM, N], f32r, kind="ExternalInput")
    wqk_d = nc.dram_tensor("w_qk", [DIM, 512], f32r, kind="ExternalInput")
    wv_d = nc.dram_tensor("w_v", [DIM, 256], f32r, kind="ExternalInput")
    bqk_d = nc.dram_tensor("b_qk", [P, 4], f32, kind="ExternalInput")
    bv_d = nc.dram_tensor("b_v", [P, 2], f32, kind="ExternalInput")
    wp_d = nc.dram_tensor("w_p", [DIM, DIM], f32r, kind="ExternalInput")
    xpb_d = nc.dram_tensor("x_pb", [TOKS, DIM], f32, kind="ExternalInput")
    wf1_d = nc.dram_tensor("w_f1", [DIM, FF], f32r, kind="ExternalInput")
    bf1_d = nc.dram_tensor("b_f1", [P, FF32], f32, kind="ExternalInput")
    wf2_d = nc.dram_tensor("w_f2", [FF, DIM], f32r, kind="ExternalInput")
    ones_d = nc.dram_tensor("ones_c", [P, 32], f32r, kind="ExternalInput")
    yout_d = nc.dram_tensor("y_out", [TOKS, DIM], f32, kind="ExternalOutput")

    def bc_row(ap_row, parts):
        # partition-broadcast AP for DMA: read one row into `parts` partitions
        t = ap_row
        dims = [list(d) for d in t.ap]
        if dims and dims[0][1] == 1:
            dims = dims[1:]
        return bass.AP(tensor=t.tensor, offset=t.offset,
                       ap=[[0, parts]] + dims)

    with tile.TileContext(nc) as tc:
        with ExitStack() as top:
            const = top.enter_context(tc.tile_pool(name="const", bufs=1))
            ones_r = const.tile([P, 32], f32r, name="ones_r")
            nc.sync.dma_start(out=ones_r, in_=ones_d.ap())
            ident = const.tile([P, P], f32, name="ident")
            make_identity(nc, ident)
            bqk_sb = const.tile([P, 4], f32, name="bqk_sb")
            nc.sync.dma_start(out=bqk_sb, in_=bqk_d.ap())
            bv_sb = const.tile([P, 2], f32, name="bv_sb")
            nc.sync.dma_start(out=bv_sb, in_=bv_d.ap())
            eps_sb = const.tile([P, 1], f32, name="eps_sb")
            nc.vector.memset(eps_sb, EPS)

            dram = top.enter_context(tc.tile_pool(name="dram", bufs=1, space="DRAM"))

            with ExitStack() as phase1:
                # phase-1 activations (freed after the AllToAll send)
                p1 = phase1.enter_context(tc.tile_pool(name="p1", bufs=1))
                # q pair tiles (2 heads stacked) + per-head zero-padded K^T:
                # khat[h] has k_h in rows (h%2)*64..+64 and zeros elsewhere, so
                # QK can contract over the full 128 partitions (K=64 matmuls
                # hold the PE at half clock - HAM sees a half-idle array).
                q_sb = [p1.tile([P, N], f32r, name=f"q{m}") for m in range(2)]
                khat = [p1.tile([P, N], f32r, name=f"kh{h}") for h in range(4)]
                v_sb = [p1.tile([P, 4, 65], f32r, name=f"v{m}") for m in range(MC16)]
                rstd_bc = p1.tile([P, N], f32, name="rstd_bc")
                rstd_col = p1.tile([P, MC16], f32, name="rstd_col")

                # ---------------- LN1 stats + qkv ----------------
                with ExitStack() as ph1:
                    xt_pool = ph1.enter_context(tc.tile_pool(name="xt", bufs=1))
                    w_pool = ph1.enter_context(tc.tile_pool(name="wqkv", bufs=1))
                    st_pool = ph1.enter_context(tc.tile_pool(name="st", bufs=1))

                    xt = []
                    for c in range(C8):
                        t = xt_pool.tile([P, N], f32r, name=f"xt{c}")
                        nc.sync.dma_start(out=t, in_=xT_d.ap()[c * P:(c + 1) * P, :])
                        xt.append(t)
                    wqk = []
                    for c in range(C8):
                        t = w_pool.tile([P, 512], f32r, name=f"wqk{c}")
                        nc.sync.dma_start(out=t, in_=wqk_d.ap()[c * P:(c + 1) * P, :])
                        wqk.append(t)
                    wv = []
                    for c in range(C8):
                        t = w_pool.tile([P, 256], f32r, name=f"wv{c}")
                        nc.sync.dma_start(out=t, in_=wv_d.ap()[c * P:(c + 1) * P, :])
                        wv.append(t)

                    # stats: S1 = sum_c x, S2 = sum_c x^2 (column sums via ones-matmul)
                    with ExitStack() as stc:
                        sps = stc.enter_context(
                            tc.tile_pool(name="sps", bufs=1, space="PSUM"))
                        s1p = [sps.tile([32, 512], f32, name=f"s1p{nt}", tag=f"s1p{nt}")
                               for nt in range(NT)]
                        s2p = [sps.tile([32, 512], f32, name=f"s2p{nt}", tag=f"s2p{nt}")
                               for nt in range(NT)]
                        for c in range(C8):
                            sq_t = st_pool.tile([P, N], f32r, name="sq", tag="sq")
                            nc.scalar.activation(out=sq_t, in_=xt[c].bitcast(f32),
                                                 func=AF.Square)
                            for nt in range(NT):
                                nsl = slice(nt * 512, (nt + 1) * 512)
                                nc.tensor.matmul(s1p[nt], ones_r, xt[c][:, nsl],
                                                 start=(c == 0), stop=(c == C8 - 1),
                                                 skip_group_check=True)
                                nc.tensor.matmul(s2p[nt], ones_r, sq_t[:, nsl],
                                                 start=(c == 0), stop=(c == C8 - 1),
                                                 skip_group_check=True)
                        # engines can only address partition bases 0/32/64/96,
                        # so stage each psum row through a small sbuf tile and
                        # bounce via DRAM into a [4, 512] layout
                        st_d = dram.tile([2, 4, 512], f32, name="st_d")
                        for nt in range(NT):
                            for k, sp in ((0, s1p[nt]), (1, s2p[nt])):
                                stg = st_pool.tile([1, 512], f32, name="stg",
                                                   tag="stg", bufs=4)
                                nc.vector.tensor_copy(stg, sp[0:1, :])
                                nc.sync.dma_start(out=st_d[k, nt], in_=stg)
                        stats1 = p1.tile([4, 512], f32, name="stats1")
                        stats2 = p1.tile([4, 512], f32, name="stats2")
                        nc.sync.dma_start(out=stats1, in_=st_d[0])
                        nc.sync.dma_start(out=stats2, in_=st_d[1])

                    # rstd = 1/sqrt(S2/C - (S1/C)^2 + eps)   on [4, 512]
                    mu4 = p1.tile([4, 512], f32, name="mu4")
                    var4 = p1.tile([4, 512], f32, name="var4")
                    nc.vector.tensor_scalar_mul(mu4, stats1, 1.0 / DIM)
                    nc.vector.tensor_scalar_mul(var4, stats2, 1.0 / DIM)
                    msq = p1.tile([4, 512], f32, name="msq")
                    nc.vector.tensor_mul(msq, mu4, mu4)
                    nc.vector.tensor_sub(var4, var4, msq)
                    sd4 = p1.tile([4, 512], f32, name="sd4")
                    nc.scalar.activation(out=sd4, in_=var4, func=AF.Sqrt,
                                         bias=eps_sb[0:4, :])
                    rstd4 = p1.tile([4, 512], f32, name="rstd4")
                    nc.vector.reciprocal(rstd4, sd4)
                    # broadcast rstd along tokens-on-free (rstd_bc) and
                    # tokens-on-partitions (rstd_col) via a DRAM bounce
                    # (partition-broadcast APs are only legal on DRAM tensors)
                    rs_d = dram.tile([MC16, P], f32, name="rs_d")
                    nc.sync.dma_start(
                        out=rs_d.rearrange("(a b) c -> a (b c)", b=4), in_=rstd4)
                    rs_flat = rs_d.rearrange("a b -> (a b)")
                    for nt in range(NT):
                        nc.sync.dma_start(
                            out=rstd_bc[:, nt * 512:(nt + 1) * 512],
                            in_=bc_row(rs_flat[nt * 512:(nt + 1) * 512], P))
                    nc.sync.dma_start(
                        out=rstd_col, in_=rs_d.rearrange("c p -> p c"))

                    # q,k feature-major: qk_sb[mi] = (W0_qk^T x)[mi]*rstd + b_qk[mi]
                    with ExitStack() as qkc:
                        qps = qkc.enter_context(
                            tc.tile_pool(name="qps", bufs=5, space="PSUM"))
                        for h in range(4):
                            nc.vector.memset(
                                khat[h].bitcast(f32)[(1 - h % 2) * 64:
                                                     (2 - h % 2) * 64, :], 0.0)
                        for mi in range(4):
                            for nt in range(NT):
                                nsl = slice(nt * 512, (nt + 1) * 512)
                                ps = qps.tile([P, 512], f32, name="qkp", tag="qkp")
                                for c in range(C8):
                                    nc.tensor.matmul(
                                        ps, wqk[c][:, mi * P:(mi + 1) * P],
                                        xt[c][:, nsl],
                                        start=(c == 0), stop=(c == C8 - 1))
                                if mi < 2:
                                    nc.vector.tensor_tensor(
                                        out=q_sb[mi][:, nsl],
                                        in0=ps, in1=rstd_bc[:, nsl], op=ALU.mult)
                                else:
                                    for hh in range(2):
                                        h = (mi - 2) * 2 + hh
                                        psl = slice(hh * 64, (hh + 1) * 64)
                                        nc.vector.tensor_tensor(
                                            out=khat[h][psl, nsl],
                                            in0=ps[psl, :],
                                            in1=rstd_bc[psl, nsl], op=ALU.mult)
                        for mi in range(2):
                            nc.vector.tensor_scalar_add(
                                q_sb[mi], q_sb[mi].bitcast(f32),
                                bqk_sb[:, mi:mi + 1])
                        for h in range(4):
                            psl = slice((h % 2) * 64, (h % 2 + 1) * 64)
                            nc.vector.tensor_scalar_add(
                                khat[h][psl, :], khat[h].bitcast(f32)[psl, :],
                                bqk_sb[psl, 2 + h // 2:3 + h // 2])
                        # v token-major: v_sb[mc] = (x^T W0_v)[mc] * rstd_col[mc]
                        for mc in range(MC16):
                            ps = qps.tile([P, 256], f32, name="vp", tag="vp", bufs=3)
                            for c in range(C8):
                                nc.tensor.matmul(ps, xt[c][:, mc * P:(mc + 1) * P],
                                                 wv[c],
                                                 start=(c == 0), stop=(c == C8 - 1))
                            nc.vector.tensor_scalar_mul(
                                v_sb[mc][:, :, 0:64],
                                ps.rearrange("p (h d) -> p h d", h=4),
                                rstd_col[:, mc:mc + 1])
                            nc.vector.memset(v_sb[mc].bitcast(f32)[:, :, 64:65], 1.0)

                # ---------------- attention ----------------
                # allocated after the xt pool closes so the slots reuse its space
                o_sb = [p1.tile([P, N], f32r, name=f"o{pr}") for pr in range(2)]
                with ExitStack() as ph2:
                    e_pool = ph2.enter_context(tc.tile_pool(name="epool", bufs=3))
                    r_pool = ph2.enter_context(tc.tile_pool(name="rpool", bufs=2))
                    aps = ph2.enter_context(
                        tc.tile_pool(name="aps", bufs=1, space="PSUM"))
                    for nt in range(NT):
                        nsl = slice(nt * 512, (nt + 1) * 512)
                        o_ps = [aps.tile([65, 512], f32, name=f"ops{h}",
                                         tag=f"ops{h}") for h in range(4)]
                        # software pipeline: AV for chunk mc-1 is issued after
                        # QK/exp for chunk mc, so the PE never waits on ACT
                        prev_e = [None] * 4
                        for mc in range(MC16 + 1):
                            if mc < MC16:
                                msl = slice(mc * P, (mc + 1) * P)
                                cur_e = []
                                for h in range(4):
                                    pr, hh = divmod(h, 2)
                                    s_ps = aps.tile([P, 512], f32, name=f"sps{h}",
                                                    tag=f"sps{h}")
                                    nc.tensor.matmul(s_ps, khat[h][:, msl],
                                                     q_sb[pr][:, nsl],
                                                     start=True, stop=True)
                                    e_t = e_pool.tile([P, 512], f32r,
                                                      name=f"e{h}", tag=f"e{h}")
                                    nc.scalar.activation(out=e_t, in_=s_ps,
                                                         func=AF.Exp,
                                                         scale=float(HD) ** -0.5)
                                    cur_e.append(e_t)
                            if mc > 0:
                                # [V_h | 1] x E_h: rows 0:64 = o_h, row 64 = denom
                                for h in range(4):
                                    nc.tensor.matmul(
                                        o_ps[h], v_sb[mc - 1][:, h, :], prev_e[h],
                                        start=(mc == 1), stop=(mc == MC16),
                                        skip_group_check=True)
                            if mc < MC16:
                                prev_e = cur_e
                        rv_d = dram.tile([4, 512], f32, name="rv_d", tag="rv_d",
                                         bufs=2)
                        for h in range(4):
                            rinv = r_pool.tile([1, 512], f32, name="rinv",
                                               tag="rinv")
                            nc.vector.reciprocal(rinv, o_ps[h][64:65, :])
                            nc.sync.dma_start(out=rv_d[h:h + 1, :], in_=rinv)
                        for h in range(4):
                            pr, hh = divmod(h, 2)
                            rbc = r_pool.tile([64, 512], f32, name=f"rbc{h % 2}",
                                              tag=f"rbc{h % 2}")
                            nc.sync.dma_start(
                                out=rbc, in_=bc_row(rv_d[h:h + 1, :], 64))
                            nc.vector.tensor_tensor(
                                out=o_sb[pr][hh * 64:(hh + 1) * 64, nsl],
                                in0=o_ps[h][0:64, :], in1=rbc, op=ALU.mult)
                    for pr in range(2):
                        nc.vector.tensor_scalar_add(
                            o_sb[pr], o_sb[pr].bitcast(f32), bv_sb[:, pr:pr + 1])

                # ---------------- AllToAll send ----------------
                # 8-core AllToAll: block j carries my 4 heads for the 256-token
                # slice of MY batch that core j will own in phase 2.
                send = dram.tile([8, 256, 256], f32r, name="a2a_send")
                for j in range(8):
                    for pr in range(2):
                        nc.sync.dma_start(
                            out=send[j, pr * P:(pr + 1) * P, :],
                            in_=o_sb[pr][:, j * 256:(j + 1) * 256])

            recv = dram.tile([8, 256, 256], f32r, name="a2a_recv")
            nc.gpsimd.collective_compute(
                "AllToAll", mybir.AluOpType.bypass,
                replica_groups=[list(range(8))],
                ins=[send.opt()], outs=[recv.opt()])

            # ---------------- phase 2: proj + residual + LN2 ----------------
            # phase-2 tokens on this core: 256 of batch 0 (cols 0:256) then
            # 256 of batch 1 (cols 256:512); head-chunk c of the gathered
            # o^T comes from group-rank c//2, pair c%2, per batch.
            p2 = top.enter_context(tc.tile_pool(name="p2", bufs=1))
            of_T = [p2.tile([P, 512], f32r, name=f"of{i}") for i in range(C8)]
            for i in range(C8):
                for g in range(2):
                    nc.sync.dma_start(
                        out=of_T[i][:, g * 256:(g + 1) * 256],
                        in_=recv[g * 4 + i // 2, (i % 2) * P:(i % 2 + 1) * P, :])
            xpb = [p2.tile([P, DIM], f32, name=f"xpb{n4}") for n4 in range(4)]
            for n4 in range(4):
                nc.sync.dma_start(out=xpb[n4],
                                  in_=xpb_d.ap()[n4 * P:(n4 + 1) * P, :])
            x2_sb = xpb  # updated in place by the proj residual add
            h2T = [p2.tile([P, TOKS], f32r, name=f"h2T{c}") for c in range(C8)]

            with ExitStack() as ph4:
                wp_pool = ph4.enter_context(tc.tile_pool(name="wp", bufs=8))
                h2_pool = ph4.enter_context(tc.tile_pool(name="h2p", bufs=2))
                pps = ph4.enter_context(tc.tile_pool(name="pps", bufs=1, space="PSUM"))
                tps = ph4.enter_context(tc.tile_pool(name="tps", bufs=2, space="PSUM"))
                for ct in range(2):
                    csl = slice(ct * 512, (ct + 1) * 512)
                    pj_ps = [pps.tile([P, 512], f32, name=f"pj{n4}", tag=f"pj{n4}")
                             for n4 in range(4)]
                    for hdc in range(C8):
                        wpt = wp_pool.tile([P, 512], f32r, name="wpt", tag="wpt")
                        nc.sync.dma_start(
                            out=wpt, in_=wp_d.ap()[hdc * P:(hdc + 1) * P, csl])
                        for n4 in range(4):
                            nc.tensor.matmul(pj_ps[n4],
                                             of_T[hdc][:, n4 * P:(n4 + 1) * P], wpt,
                                             start=(hdc == 0), stop=(hdc == C8 - 1))
                    for n4 in range(4):
                        nc.vector.tensor_tensor(out=x2_sb[n4][:, csl],
                                                in0=pj_ps[n4],
                                                in1=x2_sb[n4][:, csl],
                                                op=ALU.add)
                # LN2 (token-major, bn_stats) + transpose to h2T
                for n4 in range(4):
                    st = h2_pool.tile([P, 2, 6], f32, name="bnst", tag="bnst")
                    for g in range(2):
                        nc.vector.bn_stats(out=st[:, g, :],
                                           in_=x2_sb[n4][:, g * 512:(g + 1) * 512])
                    mv = h2_pool.tile([P, 2], f32, name="bnmv", tag="bnmv")
                    nc.vector.bn_aggr(out=mv, in_=st)
                    sd = h2_pool.tile([P, 1], f32, name="sd2", tag="sd2")
                    nc.scalar.activation(out=sd, in_=mv[:, 1:2], func=AF.Sqrt,
                                         bias=eps_sb)
                    rstd2 = h2_pool.tile([P, 1], f32, name="rstd2", tag="rstd2")
                    nc.vector.reciprocal(rstd2, sd)
                    h2t_ = h2_pool.tile([P, DIM], f32, name="h2t_", tag="h2t_")
                    nc.vector.tensor_scalar(out=h2t_, in0=x2_sb[n4],
                                            scalar1=mv[:, 0:1], scalar2=rstd2,
                                            op0=ALU.subtract, op1=ALU.mult)
                    for c in range(C8):
                        tp = tps.tile([P, P], f32, name="tp", tag="tp")
                        nc.tensor.transpose(tp, h2t_[:, c * P:(c + 1) * P], ident)
                        nc.vector.tensor_copy(h2T[c][:, n4 * P:(n4 + 1) * P], tp)

            # ---------------- MLP ----------------
            bf1_sb = p2.tile([P, FF32], f32, name="bf1_sb")
            nc.sync.dma_start(out=bf1_sb, in_=bf1_d.ap())
            g_T = [p2.tile([P, TOKS], f32r, name=f"gT{f}") for f in range(FF32)]
            yout = [p2.tile([P, DIM], f32, name=f"yo{n4}") for n4 in range(4)]
            with ExitStack() as ph5:
                w1_pool = ph5.enter_context(tc.tile_pool(name="w1", bufs=12))
                f1ps = ph5.enter_context(tc.tile_pool(name="f1ps", bufs=1,
                                                      space="PSUM"))
                for ffg in range(8):
                    f1_ps = [f1ps.tile([P, 512], f32, name=f"f1p{f}", tag=f"f1p{f}")
                             for f in range(4)]
                    for c in range(C8):
                        w1t = w1_pool.tile([P, 512], f32r, name="w1t", tag="w1t")
                        nc.sync.dma_start(
                            out=w1t, in_=wf1_d.ap()[c * P:(c + 1) * P,
                                                    ffg * 512:(ffg + 1) * 512])
                        for f in range(4):
                            nc.tensor.matmul(f1_ps[f], w1t[:, f * P:(f + 1) * P],
                                             h2T[c],
                                             start=(c == 0), stop=(c == C8 - 1))
                    for f in range(4):
                        ffc = ffg * 4 + f
                        nc.scalar.activation(out=g_T[ffc], in_=f1_ps[f],
                                             func=AF.Gelu,
                                             bias=bf1_sb[:, ffc:ffc + 1])
            with ExitStack() as ph6:
                w2_pool = ph6.enter_context(tc.tile_pool(name="w2", bufs=12))
                f2ps = ph6.enter_context(tc.tile_pool(name="f2ps", bufs=1,
                                                      space="PSUM"))
                for ct in range(2):
                    csl = slice(ct * 512, (ct + 1) * 512)
                    f2_ps = [f2ps.tile([P, 512], f32, name=f"f2p{n4}", tag=f"f2p{n4}")
                             for n4 in range(4)]
                    for ffc in range(FF32):
                        w2t = w2_pool.tile([P, 512], f32r, name="w2t", tag="w2t")
                        nc.sync.dma_start(
                            out=w2t, in_=wf2_d.ap()[ffc * P:(ffc + 1) * P, csl])
                        for n4 in range(4):
                            nc.tensor.matmul(f2_ps[n4],
                                             g_T[ffc][:, n4 * P:(n4 + 1) * P], w2t,
                                             start=(ffc == 0), stop=(ffc == FF32 - 1))
                    for n4 in range(4):
                        nc.vector.tensor_tensor(out=yout[n4][:, csl],
                                                in0=f2_ps[n4], in1=x2_sb[n4][:, csl],
                                                op=ALU.add)
            for n4 in range(4):
                nc.sync.dma_start(out=yout_d.ap()[n4 * P:(n4 + 1) * P, :],
                                  in_=yout[n4])

    nc.compile()
    _CACHE["nc"] = nc
    return nc


def _prep_inputs(x, ln1_g, ln1_b, qkv_w, proj_w, proj_b, ln2_g, ln2_b,
                 fc1_w, fc1_b, fc2_w, fc2_b):
    """Host-side sharding + weight folding. Returns list of 8 in_maps."""
    f4 = np.float32
    x = np.asarray(x, f4)
    qkv_w = np.asarray(qkv_w, f4)
    w1 = qkv_w * np.asarray(ln1_g, f4)[:, None]          # gamma fold
    w1c = w1 - w1.mean(axis=0, keepdims=True)            # mean-centering fold
    bqkv = np.asarray(ln1_b, f4) @ qkv_w                 # beta fold -> bias [3*DIM]
    w1c_r = w1c.reshape(DIM, 3, HEADS, HD)
    bqkv_r = bqkv.reshape(3, HEADS, HD)

    wf1 = np.asarray(fc1_w, f4) * np.asarray(ln2_g, f4)[:, None]
    wf1c = np.ascontiguousarray(wf1 - wf1.mean(axis=0, keepdims=True))
    bf1 = np.asarray(fc1_b, f4) + np.asarray(ln2_b, f4) @ np.asarray(fc1_w, f4)

    in_maps = []
    for core in range(NCORES):
        g, j = divmod(core, 4)
        h0 = 4 * j
        wq = w1c_r[:, 0, h0:h0 + 4, :].reshape(DIM, 256)
        wk = w1c_r[:, 1, h0:h0 + 4, :].reshape(DIM, 256)
        wv = w1c_r[:, 2, h0:h0 + 4, :].reshape(DIM, 256)
        bq = bqkv_r[0, h0:h0 + 4, :].reshape(256)
        bk = bqkv_r[1, h0:h0 + 4, :].reshape(256)
        bv = bqkv_r[2, h0:h0 + 4, :].reshape(256)
        in_maps.append({
            "xT": np.ascontiguousarray(x[g].T),
            "w_qk": np.ascontiguousarray(np.concatenate([wq, wk], axis=1)),
            "w_v": np.ascontiguousarray(wv),
            "b_qk": np.ascontiguousarray(
                np.concatenate([bq, bk]).reshape(4, P).T),
            "b_v": np.ascontiguousarray(bv.reshape(2, P).T),
            "w_p": np.asarray(proj_w, f4),
            "x_pb": np.ascontiguousarray(
                np.concatenate([x[0, core * 256:(core + 1) * 256, :],
                                x[1, core * 256:(core + 1) * 256, :]], axis=0)
                + np.asarray(proj_b, f4)),
            "w_f1": wf1c,
            "b_f1": np.ascontiguousarray(bf1.reshape(FF32, P).T),
            "w_f2": np.asarray(fc2_w, f4),
            "ones_c": np.ones((P, 32), f4),
        })
    return in_maps


def kernel(**inputs):
    from concourse.bass_utils import run_bass_kernel_spmd

    nc = _build()
    in_maps = _prep_inputs(**inputs)
    res = run_bass_kernel_spmd(nc, in_maps, core_ids=list(range(NCORES)))
    return assemble_output(res.results, inputs)


def assemble_output(results, inputs):
    fc2_b = np.asarray(inputs["fc2_b"], np.float32)
    out = np.empty((B, N, DIM), np.float32)
    for core in range(NCORES):
        y = results[core]["y_out"] + fc2_b
        out[0, core * 256:(core + 1) * 256, :] = y[0:256]
        out[1, core * 256:(core + 1) * 256, :] = y[256:512]
    return out


# revision 25
# speedup vs baseline: 1.3754x; 1.0095x over previous
"""Trainium2 Bass kernel for a pre-norm transformer block (B=2, N=2048, D=1024, H=16, FF=4096).

Strategy (8 cores):
  Phase 1 (DP on batch x TP on heads): cores 0-3 handle batch 0, cores 4-7 batch 1;
    each core computes LN1 + qkv + attention for its 4 heads over all 2048 tokens,
    in feature-major ("transposed") layout so no on-chip transposes are needed.
    LN1 is folded into the weights host-side (gamma-scaling + mean-centering of the
    qkv weight columns); the per-token rstd is applied after the matmul via a
    DMA-broadcast row. Softmax runs without max-subtraction (scores are O(10), fp32
    exp is safe); denominators come from column-packed ones-matmuls.
  Reshard: one AllToAll per 4-core group moves head-shards -> token-shards.
  Phase 2 (token-parallel): each core runs proj + residual + LN2 + MLP for its 512
    tokens with full weights. proj_b is pre-added to the residual host-side; ln2_g/b
    are folded into fc1_w/fc1_b host-side; fc2_b is added host-side after gather.

All matmuls run in float32r (full PE rate at N>=256, ~1e-3 matmul accuracy).
"""

import numpy as np

DIM = 1024
HEADS = 16
HD = 64
FF = 4096
B = 2
N = 2048
EPS = 1e-5
P = 128
NCORES = 8
GROUPS = [[0, 1, 2, 3], [4, 5, 6, 7]]
TOKS = 512         # tokens per core in phase 2
C8 = DIM // P      # 8 contraction chunks
NT = N // 512      # 4 n-tiles
MC16 = N // P      # 16 m-chunks
FF32 = FF // P     # 32 ff chunks

_CACHE = {}


def _build():
    if "nc" in _CACHE:
        return _CACHE["nc"]

    import concourse.bacc as bacc
    import concourse.bass as bass
    import concourse.tile as tile
    from concourse import mybir
    from concourse.masks import make_identity
    from contextlib import ExitStack

    f32 = mybir.dt.float32
    f32r = mybir.dt.float32r
    AF = mybir.ActivationFunctionType
    ALU = mybir.AluOpType

    nc = bacc.Bacc("TRN2", target_bir_lowering=False, debug=False,
                   num_devices=NCORES)

    # ---- per-core dram tensors ----
    xT_d = nc.dram_tensor("xT", [DIM, N], f32r, kind="ExternalInput")
    wqk_d = nc.dram_tensor("w_qk", [DIM, 512], f32r, kind="ExternalInput")
    wv_d = nc.dram_tensor("w_v", [DIM, 256], f32r, kind="ExternalInput")
    bqk_d = nc.dram_tensor("b_qk", [P, 4], f32, kind="ExternalInput")
    bv_d = nc.dram_tensor("b_v", [P, 2], f32, kind="ExternalInput")
    wp_d = nc.dram_tensor("w_p", [DIM, DIM], f32r, kind="ExternalInput")
    xpb_d = nc.dram_tensor("x_pb", [TOKS, DIM], f32, kind="ExternalInput")
    wf1_d = nc.dram_tensor("w_f1", [DIM, FF], f32r, kind="ExternalInput")
    bf1_d = nc.dram_tensor("b_f1", [P, FF32], f32, kind="ExternalInput")
    wf2_d = nc.dram_tensor("w_f2", [FF, DIM], f32r, kind="ExternalInput")
    ones_d = nc.dram_tensor("ones_c", [P, 32], f32r, kind="ExternalInput")
    yout_d = nc.dram_tensor("y_out", [TOKS, DIM], f32, kind="ExternalOutput")

    def bc_row(ap_row, parts):
        # partition-broadcast AP for DMA: read one row into `parts` partitions
        t = ap_row
        dims = [list(d) for d in t.ap]
        if dims and dims[0][1] == 1:
            dims = dims[1:]
        return bass.AP(tensor=t.tensor, offset=t.offset,
                       ap=[[0, parts]] + dims)

    with tile.TileContext(nc) as tc:
        with ExitStack() as top:
            const = top.enter_context(tc.tile_pool(name="const", bufs=1))
            ones_r = const.tile([P, 32], f32r, name="ones_r")
            nc.sync.dma_start(out=ones_r, in_=ones_d.ap())
            ident = const.tile([P, P], f32, name="ident")
            make_identity(nc, ident)
            bqk_sb = const.tile([P, 4], f32, name="bqk_sb")
            nc.sync.dma_start(out=bqk_sb, in_=bqk_d.ap())
            bv_sb = const.tile([P, 2], f32, name="bv_sb")
            nc.sync.dma_start(out=bv_sb, in_=bv_d.ap())
            eps_sb = const.tile([P, 1], f32, name="eps_sb")
            nc.vector.memset(eps_sb, EPS)

            dram = top.enter_context(tc.tile_pool(name="dram", bufs=1, space="DRAM"))

            with ExitStack() as phase1:
                # phase-1 activations (freed after the AllToAll send)
                p1 = phase1.enter_context(tc.tile_pool(name="p1", bufs=1))
                # q pair tiles (2 heads stacked) + per-head zero-padded K^T:
                # khat[h] has k_h in rows (h%2)*64..+64 and zeros elsewhere, so
                # QK can contract over the full 128 partitions (K=64 matmuls
                # hold the PE at half clock - HAM sees a half-idle array).
                q_sb = [p1.tile([P, N], f32r, name=f"q{m}") for m in range(2)]
                khat = [p1.tile([P, N], f32r, name=f"kh{h}") for h in range(4)]
                v_sb = [p1.tile([P, 4, 65], f32r, name=f"v{m}") for m in range(MC16)]
                rstd_bc = p1.tile([P, N], f32, name="rstd_bc")
                rstd_col = p1.tile([P, MC16], f32, name="rstd_col")

                # ---------------- LN1 stats + qkv ----------------
                with ExitStack() as ph1:
                    xt_pool = ph1.enter_context(tc.tile_pool(name="xt", bufs=1))
                    w_pool = ph1.enter_context(tc.tile_pool(name="wqkv", bufs=1))
                    st_pool = ph1.enter_context(tc.tile_pool(name="st", bufs=1))

                    xt = []
                    for c in range(C8):
                        t = xt_pool.tile([P, N], f32r, name=f"xt{c}")
                        nc.sync.dma_start(out=t, in_=xT_d.ap()[c * P:(c + 1) * P, :])
                        xt.append(t)
                    wqk = []
                    for c in range(C8):
                        t = w_pool.tile([P, 512], f32r, name=f"wqk{c}")
                        nc.sync.dma_start(out=t, in_=wqk_d.ap()[c * P:(c + 1) * P, :])
                        wqk.append(t)
                    wv = []
                    for c in range(C8):
                        t = w_pool.tile([P, 256], f32r, name=f"wv{c}")
                        nc.sync.dma_start(out=t, in_=wv_d.ap()[c * P:(c + 1) * P, :])
                        wv.append(t)

                    # stats: S1 = sum_c x, S2 = sum_c x^2 (column sums via ones-matmul)
                    with ExitStack() as stc:
                        sps = stc.enter_context(
                            tc.tile_pool(name="sps", bufs=1, space="PSUM"))
                        s1p = [sps.tile([32, 512], f32, name=f"s1p{nt}", tag=f"s1p{nt}")
                               for nt in range(NT)]
                        s2p = [sps.tile([32, 512], f32, name=f"s2p{nt}", tag=f"s2p{nt}")
                               for nt in range(NT)]
                        for c in range(C8):
                            sq_t = st_pool.tile([P, N], f32r, name="sq", tag="sq")
                            nc.scalar.activation(out=sq_t, in_=xt[c].bitcast(f32),
                                                 func=AF.Square)
                            for nt in range(NT):
                                nsl = slice(nt * 512, (nt + 1) * 512)
                                nc.tensor.matmul(s1p[nt], ones_r, xt[c][:, nsl],
                                                 start=(c == 0), stop=(c == C8 - 1),
                                                 skip_group_check=True)
                                nc.tensor.matmul(s2p[nt], ones_r, sq_t[:, nsl],
                                                 start=(c == 0), stop=(c == C8 - 1),
                                                 skip_group_check=True)
                        # engines can only address partition bases 0/32/64/96,
                        # so stage each psum row through a small sbuf tile and
                        # bounce via DRAM into a [4, 512] layout
                        st_d = dram.tile([2, 4, 512], f32, name="st_d")
                        for nt in range(NT):
                            for k, sp in ((0, s1p[nt]), (1, s2p[nt])):
                                stg = st_pool.tile([1, 512], f32, name="stg",
                                                   tag="stg", bufs=4)
                                nc.vector.tensor_copy(stg, sp[0:1, :])
                                nc.sync.dma_start(out=st_d[k, nt], in_=stg)
                        stats1 = p1.tile([4, 512], f32, name="stats1")
                        stats2 = p1.tile([4, 512], f32, name="stats2")
                        nc.sync.dma_start(out=stats1, in_=st_d[0])
                        nc.sync.dma_start(out=stats2, in_=st_d[1])

                    # rstd = 1/sqrt(S2/C - (S1/C)^2 + eps)   on [4, 512]
                    mu4 = p1.tile([4, 512], f32, name="mu4")
                    var4 = p1.tile([4, 512], f32, name="var4")
                    nc.vector.tensor_scalar_mul(mu4, stats1, 1.0 / DIM)
                    nc.vector.tensor_scalar_mul(var4, stats2, 1.0 / DIM)
                    msq = p1.tile([4, 512], f32, name="msq")
                    nc.vector.tensor_mul(msq, mu4, mu4)
                    nc.vector.tensor_sub(var4, var4, msq)
                    sd4 = p1.tile([4, 512], f32, name="sd4")
                    nc.scalar.activation(out=sd4, in_=var4, func=AF.Sqrt,
                                         bias=eps_sb[0:4, :])
                    rstd4 = p1.tile([4, 512], f32, name="rstd4")
                    nc.vector.reciprocal(rstd4, sd4)
                    # broadcast rstd along tokens-on-free (rstd_bc) and
                    # tokens-on-partitions (rstd_col) via a DRAM bounce
                    # (partition-broadcast APs are only legal on DRAM tensors)
                    rs_d = dram.tile([MC16, P], f32, name="rs_d")
                    nc.sync.dma_start(
                        out=rs_d.rearrange("(a b) c -> a (b c)", b=4), in_=rstd4)
                    rs_flat = rs_d.rearrange("a b -> (a b)")
                    for nt in range(NT):
                        nc.sync.dma_start(
                            out=rstd_bc[:, nt * 512:(nt + 1) * 512],
                            in_=bc_row(rs_flat[nt * 512:(nt + 1) * 512], P))
                    nc.sync.dma_start(
                        out=rstd_col, in_=rs_d.rearrange("c p -> p c"))

                    # q,k feature-major: qk_sb[mi] = (W0_qk^T x)[mi]*rstd + b_qk[mi]
                    with ExitStack() as qkc:
                        qps = qkc.enter_context(
                            tc.tile_pool(name="qps", bufs=5, space="PSUM"))
                        for h in range(4):
                            nc.vector.memset(
                                khat[h].bitcast(f32)[(1 - h % 2) * 64:
                                                     (2 - h % 2) * 64, :], 0.0)
                        for mi in range(4):
                            for nt in range(NT):
                                nsl = slice(nt * 512, (nt + 1) * 512)
                                ps = qps.tile([P, 512], f32, name="qkp", tag="qkp")
                                for c in range(C8):
                                    nc.tensor.matmul(
                                        ps, wqk[c][:, mi * P:(mi + 1) * P],
                                        xt[c][:, nsl],
                                        start=(c == 0), stop=(c == C8 - 1))
                                if mi < 2:
                                    nc.vector.tensor_tensor(
                                        out=q_sb[mi][:, nsl],
                                        in0=ps, in1=rstd_bc[:, nsl], op=ALU.mult)
                                else:
                                    for hh in range(2):
                                        h = (mi - 2) * 2 + hh
                                        psl = slice(hh * 64, (hh + 1) * 64)
                                        nc.vector.tensor_tensor(
                                            out=khat[h][psl, nsl],
                                            in0=ps[psl, :],
                                            in1=rstd_bc[psl, nsl], op=ALU.mult)
                        for mi in range(2):
                            nc.vector.tensor_scalar_add(
                                q_sb[mi], q_sb[mi].bitcast(f32),
                                bqk_sb[:, mi:mi + 1])
                        for h in range(4):
                            psl = slice((h % 2) * 64, (h % 2 + 1) * 64)
                            nc.vector.tensor_scalar_add(
                                khat[h][psl, :], khat[h].bitcast(f32)[psl, :],
                                bqk_sb[psl, 2 + h // 2:3 + h // 2])
                        # v token-major: v_sb[mc] = (x^T W0_v)[mc] * rstd_col[mc]
                        for mc in range(MC16):
                            ps = qps.tile([P, 256], f32, name="vp", tag="vp", bufs=3)
                            for c in range(C8):
                                nc.tensor.matmul(ps, xt[c][:, mc * P:(mc + 1) * P],
                                                 wv[c],
                                                 start=(c == 0), stop=(c == C8 - 1))
                            nc.vector.tensor_scalar_mul(
                                v_sb[mc][:, :, 0:64],
                                ps.rearrange("p (h d) -> p h d", h=4),
                                rstd_col[:, mc:mc + 1])
                            nc.vector.memset(v_sb[mc].bitcast(f32)[:, :, 64:65], 1.0)

                # ---------------- attention ----------------
                # allocated after the xt pool closes so the slots reuse its space
                o_sb = [p1.tile([P, N], f32r, name=f"o{pr}") for pr in range(2)]
                recv = [dram.tile([8, 256, P], f32r, name=f"a2a_recv{hf}")
                        for hf in range(2)]
                with ExitStack() as ph2:
                    e_pool = ph2.enter_context(tc.tile_pool(name="epool", bufs=3))
                    r_pool = ph2.enter_context(tc.tile_pool(name="rpool", bufs=2))
                    aps = ph2.enter_context(
                        tc.tile_pool(name="aps", bufs=1, space="PSUM"))
                    for nt in range(NT):
                        nsl = slice(nt * 512, (nt + 1) * 512)
                        o_ps = [aps.tile([65, 512], f32, name=f"ops{h}",
                                         tag=f"ops{h}") for h in range(4)]
                        # software pipeline: AV for chunk mc-1 is issued after
                        # QK/exp for chunk mc, so the PE never waits on ACT
                        prev_e = [None] * 4
                        for mc in range(MC16 + 1):
                            if mc < MC16:
                                msl = slice(mc * P, (mc + 1) * P)
                                cur_e = []
                                for h in range(4):
                                    pr, hh = divmod(h, 2)
                                    s_ps = aps.tile([P, 512], f32, name=f"sps{h}",
                                                    tag=f"sps{h}")
                                    nc.tensor.matmul(s_ps, khat[h][:, msl],
                                                     q_sb[pr][:, nsl],
                                                     start=True, stop=True)
                                    e_t = e_pool.tile([P, 512], f32r,
                                                      name=f"e{h}", tag=f"e{h}")
                                    nc.scalar.activation(out=e_t, in_=s_ps,
                                                         func=AF.Exp,
                                                         scale=float(HD) ** -0.5)
                                    cur_e.append(e_t)
                            if mc > 0:
                                # [V_h | 1] x E_h: rows 0:64 = o_h, row 64 = denom
                                for h in range(4):
                                    nc.tensor.matmul(
                                        o_ps[h], v_sb[mc - 1][:, h, :], prev_e[h],
                                        start=(mc == 1), stop=(mc == MC16),
                                        skip_group_check=True)
                            if mc < MC16:
                                prev_e = cur_e
                        rv_d = dram.tile([4, 512], f32, name="rv_d", tag="rv_d",
                                         bufs=2)
                        for h in range(4):
                            rinv = r_pool.tile([1, 512], f32, name="rinv",
                                               tag="rinv")
                            nc.vector.reciprocal(rinv, o_ps[h][64:65, :])
                            nc.sync.dma_start(out=rv_d[h:h + 1, :], in_=rinv)
                        for h in range(4):
                            pr, hh = divmod(h, 2)
                            rbc = r_pool.tile([64, 512], f32, name=f"rbc{h % 2}",
                                              tag=f"rbc{h % 2}")
                            nc.sync.dma_start(
                                out=rbc, in_=bc_row(rv_d[h:h + 1, :], 64))
                            nc.vector.tensor_tensor(
                                out=o_sb[pr][hh * 64:(hh + 1) * 64, nsl],
                                in0=o_ps[h][0:64, :], in1=rbc, op=ALU.mult)
                            nc.vector.tensor_scalar_add(
                                o_sb[pr][hh * 64:(hh + 1) * 64, nsl],
                                o_sb[pr].bitcast(f32)[hh * 64:(hh + 1) * 64, nsl],
                                bv_sb[hh * 64:(hh + 1) * 64, pr:pr + 1])
                        # AllToAll reshard, split in two so the first half
                        # overlaps the second half of attention. Core j owns
                        # tokens [128j,128j+128) (half 0) and
                        # [1024+128j, ...) (half 1) of each batch.
                        if nt == 1 or nt == 3:
                            half = nt // 2
                            send = dram.tile([8, 256, P], f32r,
                                             name=f"a2a_send{half}")
                            for j in range(8):
                                for pr in range(2):
                                    nc.sync.dma_start(
                                        out=send[j, pr * P:(pr + 1) * P, :],
                                        in_=o_sb[pr][:, half * 1024 + j * P:
                                                     half * 1024 + (j + 1) * P])
                            nc.gpsimd.collective_compute(
                                "AllToAll", mybir.AluOpType.bypass,
                                replica_groups=[list(range(8))],
                                ins=[send.opt()], outs=[recv[half].opt()])

            # ---------------- phase 2: proj + residual + LN2 ----------------
            # phase-2 local token order: [b0-half0 | b0-half1 | b1-half0 |
            # b1-half1] x 128; head-chunk c comes from group-rank c//2,
            # pair c%2, for each (batch, half).
            p2 = top.enter_context(tc.tile_pool(name="p2", bufs=1))
            of_T = [p2.tile([P, 512], f32r, name=f"of{i}") for i in range(C8)]
            for i in range(C8):
                for g in range(2):
                    for half in range(2):
                        n4 = g * 2 + half
                        nc.sync.dma_start(
                            out=of_T[i][:, n4 * P:(n4 + 1) * P],
                            in_=recv[half][g * 4 + i // 2,
                                           (i % 2) * P:(i % 2 + 1) * P, :])
            xpb = [p2.tile([P, DIM], f32, name=f"xpb{n4}") for n4 in range(4)]
            for n4 in range(4):
                nc.sync.dma_start(out=xpb[n4],
                                  in_=xpb_d.ap()[n4 * P:(n4 + 1) * P, :])
            x2_sb = xpb  # updated in place by the proj residual add
            h2T = [p2.tile([P, TOKS], f32r, name=f"h2T{c}") for c in range(C8)]

            with ExitStack() as ph4:
                wp_pool = ph4.enter_context(tc.tile_pool(name="wp", bufs=8))
                h2_pool = ph4.enter_context(tc.tile_pool(name="h2p", bufs=2))
                pps = ph4.enter_context(tc.tile_pool(name="pps", bufs=1, space="PSUM"))
                tps = ph4.enter_context(tc.tile_pool(name="tps", bufs=2, space="PSUM"))
                for ct in range(2):
                    csl = slice(ct * 512, (ct + 1) * 512)
                    pj_ps = [pps.tile([P, 512], f32, name=f"pj{n4}", tag=f"pj{n4}")
                             for n4 in range(4)]
                    for hdc in range(C8):
                        wpt = wp_pool.tile([P, 512], f32r, name="wpt", tag="wpt")
                        nc.sync.dma_start(
                            out=wpt, in_=wp_d.ap()[hdc * P:(hdc + 1) * P, csl])
                        for n4 in range(4):
                            nc.tensor.matmul(pj_ps[n4],
                                             of_T[hdc][:, n4 * P:(n4 + 1) * P], wpt,
                                             start=(hdc == 0), stop=(hdc == C8 - 1))
                    for n4 in range(4):
                        nc.vector.tensor_tensor(out=x2_sb[n4][:, csl],
                                                in0=pj_ps[n4],
                                                in1=x2_sb[n4][:, csl],
                                                op=ALU.add)
                # LN2 (token-major, bn_stats) + transpose to h2T
                for n4 in range(4):
                    st = h2_pool.tile([P, 2, 6], f32, name="bnst", tag="bnst")
                    for g in range(2):
                        nc.vector.bn_stats(out=st[:, g, :],
                                           in_=x2_sb[n4][:, g * 512:(g + 1) * 512])
                    mv = h2_pool.tile([P, 2], f32, name="bnmv", tag="bnmv")
                    nc.vector.bn_aggr(out=mv, in_=st)
                    sd = h2_pool.tile([P, 1], f32, name="sd2", tag="sd2")
                    nc.scalar.activation(out=sd, in_=mv[:, 1:2], func=AF.Sqrt,
                                         bias=eps_sb)
                    rstd2 = h2_pool.tile([P, 1], f32, name="rstd2", tag="rstd2")
                    nc.vector.reciprocal(rstd2, sd)
                    h2t_ = h2_pool.tile([P, DIM], f32, name="h2t_", tag="h2t_")
                    nc.vector.tensor_scalar(out=h2t_, in0=x2_sb[n4],
                                            scalar1=mv[:, 0:1], scalar2=rstd2,
                                            op0=ALU.subtract, op1=ALU.mult)
                    for c in range(C8):
                        tp = tps.tile([P, P], f32, name="tp", tag="tp")
                        nc.tensor.transpose(tp, h2t_[:, c * P:(c + 1) * P], ident)
                        nc.vector.tensor_copy(h2T[c][:, n4 * P:(n4 + 1) * P], tp)

            # ---------------- MLP ----------------
            bf1_sb = p2.tile([P, FF32], f32, name="bf1_sb")
            nc.sync.dma_start(out=bf1_sb, in_=bf1_d.ap())
            g_T = [p2.tile([P, TOKS], f32r, name=f"gT{f}") for f in range(FF32)]
            yout = [p2.tile([P, DIM], f32, name=f"yo{n4}") for n4 in range(4)]
            with ExitStack() as ph5:
                w1_pool = ph5.enter_context(tc.tile_pool(name="w1", bufs=12))
                f1ps = ph5.enter_context(tc.tile_pool(name="f1ps", bufs=1,
                                                      space="PSUM"))
                for ffg in range(8):
                    f1_ps = [f1ps.tile([P, 512], f32, name=f"f1p{f}", tag=f"f1p{f}")
                             for f in range(4)]
                    for c in range(C8):
                        w1t = w1_pool.tile([P, 512], f32r, name="w1t", tag="w1t")
                        nc.sync.dma_start(
                            out=w1t, in_=wf1_d.ap()[c * P:(c + 1) * P,
                                                    ffg * 512:(ffg + 1) * 512])
                        for f in range(4):
                            nc.tensor.matmul(f1_ps[f], w1t[:, f * P:(f + 1) * P],
                                             h2T[c],
                                             start=(c == 0), stop=(c == C8 - 1))
                    for f in range(4):
                        ffc = ffg * 4 + f
                        nc.scalar.activation(out=g_T[ffc], in_=f1_ps[f],
                                             func=AF.Gelu,
                                             bias=bf1_sb[:, ffc:ffc + 1])
            with ExitStack() as ph6:
                w2_pool = ph6.enter_context(tc.tile_pool(name="w2", bufs=12))
                f2ps = ph6.enter_context(tc.tile_pool(name="f2ps", bufs=1,
                                                      space="PSUM"))
                for ct in range(2):
                    csl = slice(ct * 512, (ct + 1) * 512)
                    f2_ps = [f2ps.tile([P, 512], f32, name=f"f2p{n4}", tag=f"f2p{n4}")
                             for n4 in range(4)]
                    for ffc in range(FF32):
                        w2t = w2_pool.tile([P, 512], f32r, name="w2t", tag="w2t")
                        nc.sync.dma_start(
                            out=w2t, in_=wf2_d.ap()[ffc * P:(ffc + 1) * P, csl])
                        for n4 in range(4):
                            nc.tensor.matmul(f2_ps[n4],
                                             g_T[ffc][:, n4 * P:(n4 + 1) * P], w2t,
                                             start=(ffc == 0), stop=(ffc == FF32 - 1))
                    for n4 in range(4):
                        nc.vector.tensor_tensor(out=yout[n4][:, csl],
                                                in0=f2_ps[n4], in1=x2_sb[n4][:, csl],
                                                op=ALU.add)
            for n4 in range(4):
                nc.sync.dma_start(out=yout_d.ap()[n4 * P:(n4 + 1) * P, :],
                                  in_=yout[n4])

    nc.compile()
    _CACHE["nc"] = nc
    return nc


def _prep_inputs(x, ln1_g, ln1_b, qkv_w, proj_w, proj_b, ln2_g, ln2_b,
                 fc1_w, fc1_b, fc2_w, fc2_b):
    """Host-side sharding + weight folding. Returns list of 8 in_maps."""
    f4 = np.float32
    x = np.asarray(x, f4)
    qkv_w = np.asarray(qkv_w, f4)
    w1 = qkv_w * np.asarray(ln1_g, f4)[:, None]          # gamma fold
    w1c = w1 - w1.mean(axis=0, keepdims=True)            # mean-centering fold
    bqkv = np.asarray(ln1_b, f4) @ qkv_w                 # beta fold -> bias [3*DIM]
    w1c_r = w1c.reshape(DIM, 3, HEADS, HD)
    bqkv_r = bqkv.reshape(3, HEADS, HD)

    wf1 = np.asarray(fc1_w, f4) * np.asarray(ln2_g, f4)[:, None]
    wf1c = np.ascontiguousarray(wf1 - wf1.mean(axis=0, keepdims=True))
    bf1 = np.asarray(fc1_b, f4) + np.asarray(ln2_b, f4) @ np.asarray(fc1_w, f4)

    in_maps = []
    for core in range(NCORES):
        g, j = divmod(core, 4)
        h0 = 4 * j
        wq = w1c_r[:, 0, h0:h0 + 4, :].reshape(DIM, 256)
        wk = w1c_r[:, 1, h0:h0 + 4, :].reshape(DIM, 256)
        wv = w1c_r[:, 2, h0:h0 + 4, :].reshape(DIM, 256)
        bq = bqkv_r[0, h0:h0 + 4, :].reshape(256)
        bk = bqkv_r[1, h0:h0 + 4, :].reshape(256)
        bv = bqkv_r[2, h0:h0 + 4, :].reshape(256)
        in_maps.append({
            "xT": np.ascontiguousarray(x[g].T),
            "w_qk": np.ascontiguousarray(np.concatenate([wq, wk], axis=1)),
            "w_v": np.ascontiguousarray(wv),
            "b_qk": np.ascontiguousarray(
                np.concatenate([bq, bk]).reshape(4, P).T),
            "b_v": np.ascontiguousarray(bv.reshape(2, P).T),
            "w_p": np.asarray(proj_w, f4),
            "x_pb": np.ascontiguousarray(
                np.concatenate(
                    [x[0, core * 128:(core + 1) * 128, :],
                     x[0, 1024 + core * 128:1024 + (core + 1) * 128, :],
                     x[1, core * 128:(core + 1) * 128, :],
                     x[1, 1024 + core * 128:1024 + (core + 1) * 128, :]],
                    axis=0) + np.asarray(proj_b, f4)),
            "w_f1": wf1c,
            "b_f1": np.ascontiguousarray(bf1.reshape(FF32, P).T),
            "w_f2": np.asarray(fc2_w, f4),
            "ones_c": np.ones((P, 32), f4),
        })
    return in_maps


def kernel(**inputs):
    from concourse.bass_utils import run_bass_kernel_spmd

    nc = _build()
    in_maps = _prep_inputs(**inputs)
    res = run_bass_kernel_spmd(nc, in_maps, core_ids=list(range(NCORES)))
    return assemble_output(res.results, inputs)


def assemble_output(results, inputs):
    fc2_b = np.asarray(inputs["fc2_b"], np.float32)
    out = np.empty((B, N, DIM), np.float32)
    for core in range(NCORES):
        y = results[core]["y_out"] + fc2_b
        out[0, core * 128:(core + 1) * 128, :] = y[0:128]
        out[0, 1024 + core * 128:1024 + (core + 1) * 128, :] = y[128:256]
        out[1, core * 128:(core + 1) * 128, :] = y[256:384]
        out[1, 1024 + core * 128:1024 + (core + 1) * 128, :] = y[384:512]
    return out
